# revision 43
# baseline (speedup 1.0000x reference)
"""Trainium2 Bass kernel for a 3-layer TransformerConv GNN (nn_EncoderTransformerConv).

Strategy (8 NeuronCores, SPMD, identical instruction stream per core):
  - Nodes are partitioned across cores (6250 real nodes/core, padded to 6272 =
    49 blocks of 128). Edges are partitioned by dst core.
  - Per layer:
      Phase A  : every core computes the k|v projections for ALL nodes (replicated
                 dense bf16 matmuls) and writes a row-major bf16 kv table
                 [50176, 256] to DRAM (split lo/hi so phase B can start early);
                 it also computes q (bf16, SBUF-resident) and the root-skip s for
                 its OWN nodes.
      Phase B  : per-edge work. Edges (grouped into 128-node dst blocks, split
                 into two groups by src-table half so gather indices fit int16)
                 are processed in chunks: dma_gather of kv[src] rows; q is
                 broadcast to edges with a one-hot matmul (host-precomputed S^T);
                 vector ops compute alpha = <q,k>/8, exp, ex*v; a one-hot matmul
                 (host S) performs the segmented softmax-denominator and message
                 reductions into PSUM per dst block.
      Epilogue : normalize by the segment denominator, mean heads, add skip, relu.
      Collective: AllGather of each core's h^T block (bf16) so the next layer can
                 recompute the kv table from the full h.
  - Softmax max-subtraction is skipped: alpha = q.k/8 is bounded (|alpha| < 3 for
    these weight scales); exp() is safe.
"""
import os
import sys

sys.path.insert(0, "/opt/trn_rl_repo")

import ml_dtypes
import numpy as np

import concourse.bass as bass
import concourse.bacc as bacc
import concourse.mybir as mybir
import concourse.tile as tile
from concourse import bass_utils, library_config
from concourse.masks import make_identity

F32 = mybir.dt.float32
BF16 = mybir.dt.bfloat16
I16 = mybir.dt.int16
AF = mybir.ActivationFunctionType
OP = mybir.AluOpType
BNP = ml_dtypes.bfloat16

# ----- problem dims (hardcoded per spec) -----
SPEC = dict(N=50000, E=800000, D_IN=128, HID=64, H=2, M=8)
TILE_E = 128          # edges per tile
CHUNK_T = 20          # max tiles per edge-phase chunk (whole dst-block groups)
WARM_CH = 4           # leading chunks per layer gathered in full (buffer warmup)
BLK = 128             # dst nodes per block


def _derive(cfg):
    d = dict(cfg)
    d["C"] = d["HID"]
    d["F"] = d["H"] * d["C"]            # 128 = q/k/v width
    d["WC"] = 3 * d["F"] + d["HID"]     # 448 packed q|k|v|s
    d["KV"] = 2 * d["F"]                # 256
    d["NPC_REAL"] = d["N"] // d["M"]
    d["NBLK"] = -(-d["NPC_REAL"] // BLK)
    d["NPC"] = d["NBLK"] * BLK
    d["NPAD"] = d["M"] * d["NPC"]
    d["HALF"] = d["NPAD"] // 2
    pa = 1
    for c in range(1, 9):
        if d["NBLK"] % c == 0:
            pa = c
    d["PA_CHUNK"] = pa
    d["RHSW"] = d["H"] + d["F"]         # 130 = denom cols + exv cols
    return d


def _wrap_idx(a):
    """[M, n] int -> wrapped idx layout [M, 128, n//16] (16-partition wrap,
    replicated to 8 GPSIMD core groups)."""
    Mn, n = a.shape
    w = a.reshape(Mn, n // 16, 16).transpose(0, 2, 1)
    return np.ascontiguousarray(np.tile(w, (1, 8, 1))).astype(np.int16)


def _prep(x, edge_index, weights, d):
    """Host-side preprocessing -> (in_maps, meta). weights: dict L -> (W_all[d,WC], b_all[WC])."""
    M, NPC_REAL, NPC, NPAD, HALF, NBLK = (
        d["M"], d["NPC_REAL"], d["NPC"], d["NPAD"], d["HALF"], d["NBLK"])
    N, D_IN = d["N"], d["D_IN"]

    src = np.asarray(edge_index[0]).astype(np.int64)
    dst = np.asarray(edge_index[1]).astype(np.int64)
    core = dst // NPC_REAL
    dst_l = dst - core * NPC_REAL
    blk = dst_l // BLK
    src_p = (src // NPC_REAL) * NPC + (src % NPC_REAL)
    half = (src_p >= HALF).astype(np.int64)

    counts = np.zeros((M, 2, NBLK), np.int64)
    np.add.at(counts, (core, half, blk), 1)
    tiles = np.maximum(1, -(-counts.max(axis=0) // TILE_E))  # [2, NBLK]
    flat_tiles = tiles.reshape(-1)
    tile_off = np.concatenate([[0], np.cumsum(flat_tiles)])
    TT = int(tile_off[-1])

    # stable-sort edges by (core, half, blk); rank within group
    key = (core * 2 + half) * NBLK + blk
    order = np.argsort(key, kind="stable")
    sk = key[order]
    new_run = np.ones(len(sk), bool)
    new_run[1:] = sk[1:] != sk[:-1]
    run_idx = np.cumsum(new_run) - 1
    starts = np.nonzero(new_run)[0]
    rank = np.arange(len(sk)) - starts[run_idx]
    grp = (half * NBLK + blk)[order]
    pos = tile_off[grp] * TILE_E + rank  # position within the core's edge array
    corev = core[order]

    kv_idx = np.zeros((M, TT * TILE_E), np.int64)
    dloc = np.full((M, TT * TILE_E), -1, np.int64)
    kv_idx[corev, pos] = (src_p - half * HALF)[order]
    dloc[corev, pos] = (dst_l - blk * BLK)[order]
    assert kv_idx.max() < 2 ** 15

    S = np.zeros((M, 128, TT * BLK), BNP)
    ST = np.zeros((M, 128, TT * BLK), BNP)
    dd = dloc.reshape(M, TT, TILE_E)
    mm, tt, pp = np.nonzero(dd >= 0)
    dv = dd[mm, tt, pp]
    S[mm, pp, tt * BLK + dv] = 1.0
    ST[mm, dv, tt * BLK + pp] = 1.0

    kv_w = _wrap_idx(kv_idx)

    # node features, transposed + padded (bf16)
    xT_pad = np.zeros((D_IN, NPAD), np.float32)
    n_ids = np.arange(N)
    pid = (n_ids // NPC_REAL) * NPC + (n_ids % NPC_REAL)
    xT_pad[:, pid] = np.asarray(x).T
    xT = xT_pad.astype(BNP)

    wt = {}
    for L in (1, 2, 3):
        W_all, b_all = weights[L]   # packed k|v|q|s by _weights_from_inputs
        if L == 1:
            wt["W1"] = W_all.astype(BNP)
            wt["brep1"] = np.ascontiguousarray(
                np.tile(b_all[None, :], (128, 1)).astype(np.float32))
        else:
            wt[f"W{L}"] = np.concatenate(
                [W_all, b_all[None, :]], 0).astype(BNP)

    in_maps = []
    for m in range(M):
        im = dict(
            xT=np.ascontiguousarray(xT),
            xoT=np.ascontiguousarray(xT[:, m * NPC:(m + 1) * NPC]),
            kvidx=np.ascontiguousarray(kv_w[m]),
            S_in=np.ascontiguousarray(S[m]),
            ST_in=np.ascontiguousarray(ST[m]),
            **wt,
        )
        in_maps.append(im)

    # tile metadata: (half, blk, start, stop)
    meta = []
    groups = []  # (f, b, t0, T)
    t = 0
    for f in (0, 1):
        for b in range(NBLK):
            T = int(tiles[f, b])
            groups.append((f, b, t, T))
            t += T
            for i in range(T):
                meta.append((f, b, i == 0, i == T - 1))
    # chunks = runs of whole groups (same half), <= CHUNK_T tiles, so each
    # chunk ends exactly at a group boundary (trailing pads are call-trailing)
    chunks = []
    gi = 0
    while gi < len(groups):
        f, b, t0, T = groups[gi]
        nt = T
        gj = gi + 1
        while (gj < len(groups) and groups[gj][0] == f
               and nt + groups[gj][3] <= CHUNK_T):
            nt += groups[gj][3]
            gj += 1
        chunks.append((t0, nt, f, groups[gj - 1][1]))  # last block of chunk
        gi = gj

    # per-(core, chunk) valid gather counts: for chunks beyond the warmup,
    # the final group's padding slots get idx -1 (skipped by the gather)
    nch = len(chunks)
    gcnt = np.zeros((M, nch), np.int32)
    for j, (t0, nt, f, b_last) in enumerate(chunks):
        n = nt * TILE_E
        if j < WARM_CH:
            gcnt[:, j] = n
            continue
        Tg = int(tiles[f, b_last])
        g_t0 = t0 + nt - Tg          # first tile of the final group
        for m in range(M):
            cm = int(counts[m, f, b_last])
            pad = Tg * TILE_E - cm
            gcnt[m, j] = n - pad
            if pad:
                kv_idx_w_set_m1(kv_w, m, g_t0 * TILE_E + cm, (t0 + nt) * TILE_E)
    for m in range(M):
        in_maps[m]["kvidx"] = np.ascontiguousarray(kv_w[m])
        in_maps[m]["gcnt"] = np.ascontiguousarray(gcnt[m:m + 1])
    return in_maps, dict(TT=TT, meta=meta, chunks=chunks, tiles=tiles)


def kv_idx_w_set_m1(kv_w, m, lo, hi):
    """Set wrapped-index positions [lo, hi) to -1 for core m."""
    js = np.arange(lo, hi)
    rows = (np.arange(8) * 16)[:, None] + (js % 16)[None, :]
    cols = np.broadcast_to(js // 16, rows.shape)
    kv_w[m, rows, cols] = -1


def build_module(d, meta):
    TT, chunks, tmeta = meta["TT"], meta["chunks"], meta["meta"]
    M, NPC, NPAD, HALF, NBLK, PA_CHUNK = (
        d["M"], d["NPC"], d["NPAD"], d["HALF"], d["NBLK"], d["PA_CHUNK"])
    D_IN, F, KV, WC, HID, H, C, RHSW = (
        d["D_IN"], d["F"], d["KV"], d["WC"], d["HID"], d["H"], d["C"], d["RHSW"])
    NT_ALL = NPAD // 128          # node tiles, all cores
    NCH_ALL = NT_ALL // PA_CHUNK  # phase A chunks
    RANK_CH = NBLK // PA_CHUNK    # chunks per rank slab
    CA = (3 * RANK_CH) // 4       # rank-slab chunks covered by first collective
    SPLIT_B = CA * PA_CHUNK       # blocks covered by first collective
    COLA = SPLIT_B * 128

    nc = bacc.Bacc("TRN2", target_bir_lowering=False, debug=False, num_devices=M)
    inp = {}
    for name, shape, dt in [
        ("xT", [D_IN, NPAD], BF16), ("xoT", [D_IN, NPC], BF16),
        ("W1", [D_IN, WC], BF16), ("brep1", [128, WC], F32),
        ("W2", [HID + 1, WC], BF16), ("W3", [HID + 1, WC], BF16),
        ("kvidx", [128, TT * 8], I16),
        ("gcnt", [1, len(chunks)], mybir.dt.int32),
        ("S_in", [128, TT * BLK], BF16), ("ST_in", [128, TT * BLK], BF16),
    ]:
        inp[name] = nc.dram_tensor(name, shape, dt, kind="ExternalInput")
    h_out = nc.dram_tensor("h_out", [NPC, HID], F32, kind="ExternalOutput")

    with tile.TileContext(nc) as tc:
        with tc.tile_pool(name="dram", bufs=1, space="DRAM") as dram, \
             tc.tile_pool(name="res", bufs=1) as res:
            kv_lo = dram.tile([HALF, KV], BF16)
            kv_hi = dram.tile([NPAD - HALF, KV], BF16)
            if CA > 0:
                coll_inA = dram.tile([HID + 1, COLA], BF16)
                coll_outA = dram.tile([M * (HID + 1), COLA], BF16)
            coll_inB = dram.tile([HID + 1, NPC - COLA], BF16)
            coll_outB = dram.tile([M * (HID + 1), NPC - COLA], BF16)

            nc.gpsimd.load_library(library_config.mlp)

            # resident SBUF
            W1_sb = res.tile([D_IN, WC], BF16)
            brep1_sb = res.tile([128, WC], F32)
            W2_sb = res.tile([HID + 1, WC], BF16)
            W3_sb = res.tile([HID + 1, WC], BF16)
            kvidx_sb = res.tile([128, TT * 8], I16)
            gcnt_sb = res.tile([1, len(chunks)], mybir.dt.int32)
            q_sb = res.tile([128, NBLK * F], BF16)
            s_sb = res.tile([128, NBLK * HID], F32)
            hTown = res.tile([HID + 1, NPC], BF16)
            partA = res.tile([128, NBLK * RHSW], F32)
            ident = res.tile([128, 128], F32)
            eps2 = res.tile([128, H], F32)

            for sb, t in ((W1_sb, "W1"), (brep1_sb, "brep1"), (W2_sb, "W2"),
                          (W3_sb, "W3"), (kvidx_sb, "kvidx"), (gcnt_sb, "gcnt")):
                nc.sync.dma_start(sb[:], inp[t].ap())
            make_identity(nc, ident[:])
            nc.vector.memset(hTown[HID:HID + 1, :], 1.0)
            nc.vector.memset(eps2[:], H * 1e-16)

            for layer in (1, 2, 3):
                W_sb = {1: None, 2: W2_sb, 3: W3_sb}[layer]

                # ---------- Phase A-kv: kv table for ALL nodes ----------
                with tc.tile_pool(name="pa", bufs=4) as pa, \
                     tc.tile_pool(name="pap", bufs=4, space="PSUM") as pap:

                    # own q/s first (cheap), so phase B's first blocks unblock
                    for ch in range(RANK_CH):
                        cols = slice(ch * PA_CHUNK * 128, (ch + 1) * PA_CHUNK * 128)
                        if layer == 1:
                            la = pa.tile([D_IN, PA_CHUNK * 128], BF16, tag="la")
                            nc.sync.dma_start(la[:], inp["xoT"].ap()[:, cols])
                        for t in range(PA_CHUNK):
                            gt = ch * PA_CHUNK + t  # own node tile index
                            ps = pap.tile([128, F + HID], F32, tag="psqs",
                                          name="psqs")
                            if layer == 1:
                                nc.tensor.matmul(ps[:], la[:, t * 128:(t + 1) * 128],
                                                 W1_sb[:, KV:WC],
                                                 start=True, stop=True)
                                nc.vector.tensor_tensor(
                                    q_sb[:, gt * F:(gt + 1) * F], ps[:, 0:F],
                                    brep1_sb[:, KV:KV + F], op=OP.add)
                                nc.vector.tensor_tensor(
                                    s_sb[:, gt * HID:(gt + 1) * HID],
                                    ps[:, F:F + HID],
                                    brep1_sb[:, KV + F:WC], op=OP.add)
                            else:
                                nc.tensor.matmul(
                                    ps[:],
                                    hTown[:, gt * 128:(gt + 1) * 128],
                                    W_sb[:, KV:WC], start=True, stop=True)
                                nc.scalar.copy(q_sb[:, gt * F:(gt + 1) * F],
                                               ps[:, 0:F])
                                nc.vector.tensor_copy(
                                    s_sb[:, gt * HID:(gt + 1) * HID],
                                    ps[:, F:F + HID])

                    # kv for all nodes. Order: lo table half first; within a
                    # half, chunks covered by the first (already-landed)
                    # collective before chunks needing the second one.
                    if layer == 1:
                        ch_list = list(range(NCH_ALL))
                    else:
                        ch_list = []
                        for rg in (range(0, M // 2), range(M // 2, M)):
                            for cg in (range(0, CA), range(CA, RANK_CH)):
                                for r in rg:
                                    for c in cg:
                                        ch_list.append(r * RANK_CH + c)
                    for ch in ch_list:
                        cols = slice(ch * PA_CHUNK * 128, (ch + 1) * PA_CHUNK * 128)
                        if layer == 1:
                            la = pa.tile([D_IN, PA_CHUNK * 128], BF16, tag="la")
                            nc.sync.dma_start(la[:], inp["xT"].ap()[:, cols])
                        else:
                            r, c = ch // RANK_CH, ch % RANK_CH
                            la = pa.tile([HID + 1, PA_CHUNK * 128], BF16, tag="la")
                            if c < CA:
                                src = coll_outA[r * (HID + 1):(r + 1) * (HID + 1),
                                                c * PA_CHUNK * 128:
                                                (c + 1) * PA_CHUNK * 128]
                            else:
                                src = coll_outB[r * (HID + 1):(r + 1) * (HID + 1),
                                                (c - CA) * PA_CHUNK * 128:
                                                (c - CA + 1) * PA_CHUNK * 128]
                            nc.sync.dma_start(la[:], src)
                        kvst = pa.tile([128, PA_CHUNK * KV], BF16, tag="kvst")
                        for t in range(PA_CHUNK):
                            ps = pap.tile([128, KV], F32, tag="pskv", name="pskv")
                            dstp = kvst[:, t * KV:(t + 1) * KV]
                            if layer == 1:
                                nc.tensor.matmul(ps[:], la[:, t * 128:(t + 1) * 128],
                                                 W1_sb[:, 0:KV],
                                                 start=True, stop=True)
                                nc.vector.tensor_tensor(
                                    dstp, ps[:], brep1_sb[:, 0:KV], op=OP.add)
                            else:
                                nc.tensor.matmul(ps[:], la[:, t * 128:(t + 1) * 128],
                                                 W_sb[:, 0:KV], start=True, stop=True)
                                if t % 2 == 0:
                                    nc.scalar.copy(dstp, ps[:])
                                else:
                                    nc.vector.tensor_copy(dstp, ps[:])
                        row0 = ch * PA_CHUNK * 128
                        tgt = (kv_lo[row0:row0 + PA_CHUNK * 128, :]
                               if row0 < HALF else
                               kv_hi[row0 - HALF:row0 - HALF + PA_CHUNK * 128, :])
                        nc.sync.dma_start(
                            tgt.rearrange("(t p) e -> p t e", p=128),
                            kvst[:].rearrange("p (t e) -> p t e", e=KV))

                # ---------- Phase B: edges ----------
                with tc.tile_pool(name="pb", bufs=4) as pb, \
                     tc.tile_pool(name="pb1", bufs=3) as pb1, \
                     tc.tile_pool(name="pbp", bufs=3, space="PSUM") as pbp, \
                     tc.tile_pool(name="qep", bufs=3, space="PSUM") as qep, \
                     tc.tile_pool(name="epp", bufs=1, space="PSUM") as epp, \
                     tc.tile_pool(name="ep", bufs=2) as ep:
                    psum_blk = {}
                    gregs = [nc.gpsimd.alloc_register(f"gcnt_l{layer}_{i}")
                             for i in range(4)]
                    for j, (t0, nt, fhalf, _blast) in enumerate(chunks):
                        n = nt * TILE_E
                        if j < WARM_CH:
                            nreg = n
                        else:
                            nreg = gregs[j % 4]
                            nc.gpsimd.reg_load(nreg, gcnt_sb[0:1, j:j + 1])
                        kvg = pb.tile([128, CHUNK_T, KV], BF16, tag="kvg")
                        Sg = pb1.tile([128, CHUNK_T * BLK], BF16, tag="Sg")
                        STg = pb1.tile([128, CHUNK_T * BLK], BF16, tag="STg")
                        prod = pb1.tile([128, CHUNK_T * F], F32, tag="prod", bufs=2)
                        alph = pb1.tile([128, CHUNK_T * H], F32, tag="alph")
                        rhs = pb.tile([128, CHUNK_T, RHSW], BF16, tag="rhs")

                        in_ap = kv_lo[:] if fhalf == 0 else kv_hi[:]
                        nc.gpsimd.dma_gather(
                            out_ap=kvg[:, 0:nt, :], in_ap=in_ap,
                            idxs_ap=kvidx_sb[:, t0 * 8:t0 * 8 + nt * 8],
                            num_idxs=n, num_idxs_reg=nreg, elem_size=KV,
                            single_packet=False)
                        nc.sync.dma_start(
                            Sg[:, 0:n], inp["S_in"].ap()[:, t0 * BLK:t0 * BLK + n])
                        nc.sync.dma_start(
                            STg[:, 0:n], inp["ST_in"].ap()[:, t0 * BLK:t0 * BLK + n])

                        # q_edges per tile via one-hot broadcast matmul, then
                        # prod = (q_edges * 1/sqrt(C)) * k
                        for i in range(nt):
                            tg = t0 + i
                            b = tmeta[tg][1]
                            qe = qep.tile([128, F], F32, name="qe", tag="qe")
                            nc.tensor.matmul(
                                qe[:], STg[:, i * BLK:(i + 1) * BLK],
                                q_sb[:, b * F:(b + 1) * F], start=True, stop=True)
                            nc.vector.scalar_tensor_tensor(
                                out=prod[:, i * F:(i + 1) * F],
                                in0=qe[:], scalar=float(1.0 / np.sqrt(C)),
                                in1=kvg[:, i, 0:F], op0=OP.mult, op1=OP.mult)
                        # alpha[p, t, h] = sum_c prod
                        nc.vector.reduce_sum(
                            alph[:, 0:nt * H].rearrange("p (t h) -> p t h", h=H),
                            prod[:, 0:nt * F].rearrange(
                                "p (t h c) -> p t h c", h=H, c=C),
                            axis=mybir.AxisListType.X)
                        # ex = exp(alpha) -> rhs[:, :, 0:H]
                        nc.scalar.activation(
                            rhs[:, 0:nt, 0:H],
                            alph[:, 0:nt * H].rearrange("p (t h) -> p t h", h=H),
                            AF.Exp)
                        # exv = v * ex -> rhs[:, :, H:]
                        nc.vector.tensor_tensor(
                            out=rhs[:, 0:nt, H:RHSW].rearrange(
                                "p t (h c) -> p t h c", c=C),
                            in0=kvg[:, 0:nt, F:KV].rearrange(
                                "p t (h c) -> p t h c", c=C),
                            in1=rhs[:, 0:nt, 0:H].to_broadcast([128, nt, H, C]),
                            op=OP.mult)

                        for i in range(nt):
                            tg = t0 + i
                            f, b, st, sp = tmeta[tg]
                            if st:
                                psum_blk[(f, b)] = pbp.tile(
                                    [128, RHSW], F32, name="pblk", tag="pblk")
                            nc.tensor.matmul(
                                psum_blk[(f, b)][:],
                                Sg[:, i * BLK:(i + 1) * BLK],
                                rhs[:, i, :], start=st, stop=sp)
                            if not sp:
                                continue
                            ps = psum_blk.pop((f, b))
                            pa_sl = partA[:, b * RHSW:(b + 1) * RHSW]
                            if f == 0:
                                nc.scalar.copy(pa_sl, ps[:])
                                continue
                            # ---- epilogue for block b ----
                            tot = ep.tile([128, RHSW], F32, tag="tot")
                            nc.vector.tensor_tensor(tot[:], ps[:], pa_sl, op=OP.add)
                            # rec = (1/H) / (denom + 1e-16), via 1/(H*denom + H*1e-16)
                            rec = ep.tile([128, H], F32, tag="rec")
                            nc.vector.scalar_tensor_tensor(
                                out=rec[:], in0=tot[:, 0:H], scalar=float(H),
                                in1=eps2[:], op0=OP.mult, op1=OP.add)
                            nc.vector.reciprocal(rec[:], rec[:])
                            m0 = ep.tile([128, C], F32, tag="m0")
                            nc.vector.scalar_tensor_tensor(
                                out=m0[:], in0=tot[:, H:H + C],
                                scalar=rec[:, 0:1],
                                in1=s_sb[:, b * HID:(b + 1) * HID],
                                op0=OP.mult, op1=OP.add)
                            hp2 = ep.tile([128, HID], F32, tag="hp2")
                            nc.vector.scalar_tensor_tensor(
                                out=hp2[:], in0=tot[:, H + C:H + 2 * C],
                                scalar=rec[:, 1:2], in1=m0[:],
                                op0=OP.mult, op1=OP.add)
                            hblk = ep.tile([128, HID], F32, tag="hblk")
                            nc.scalar.activation(hblk[:], hp2[:], AF.Relu)
                            if layer < 3:
                                pst = epp.tile([HID, 128], F32)
                                nc.tensor.transpose(pst[:], hblk[:], ident[:])
                                nc.vector.tensor_copy(
                                    hTown[0:HID, b * 128:(b + 1) * 128], pst[:])
                                if CA > 0 and b == SPLIT_B - 1:
                                    # first half of h^T is final: allgather it
                                    # while the rest of phase B runs
                                    nc.sync.dma_start(coll_inA[:, :],
                                                      hTown[:, 0:COLA])
                                    nc.gpsimd.collective_compute(
                                        "AllGather", OP.bypass,
                                        ins=[coll_inA.opt()],
                                        outs=[coll_outA.opt()],
                                        replica_groups=[list(range(M))])
                            else:
                                nc.sync.dma_start(
                                    h_out.ap()[b * 128:(b + 1) * 128, :], hblk[:])
                    assert not psum_blk

                if layer < 3:
                    nc.sync.dma_start(coll_inB[:, :], hTown[:, COLA:])
                    nc.gpsimd.collective_compute(
                        "AllGather", OP.bypass,
                        ins=[coll_inB.opt()], outs=[coll_outB.opt()],
                        replica_groups=[list(range(M))])
    nc.compile()
    return nc


# ---------------- public entry ----------------
_CACHE = {}


def _weights_from_inputs(inputs, d):
    # packed column order: k | v | q | s
    wt = {}
    for L in (1, 2, 3):
        W_all = np.concatenate(
            [np.asarray(inputs[f"W{L}{nm}"], np.float32) for nm in ("k", "v", "q", "s")],
            axis=1)
        b_all = np.concatenate(
            [np.asarray(inputs[f"b{L}{nm}"], np.float32) for nm in ("k", "v", "q", "s")])
        wt[L] = (W_all, b_all)
    return wt


def _install_ntff_shim():
    import types
    if "antenv.axon_hooks" in sys.modules:
        return
    try:
        from trn_agent_boot.trn_boot import _ntff_profile_via_ctypes
        hook = _ntff_profile_via_ctypes("/opt/axon/libaxon_pjrt.so")
    except Exception:
        hook = None
    mod = types.ModuleType("antenv.axon_hooks")
    mod.get_axon_ntff_profile_hook = lambda: hook
    mod.set_axon_ntff_profile_hook = lambda h: None
    sys.modules["antenv.axon_hooks"] = mod
    try:
        import antenv
        antenv.axon_hooks = mod
    except Exception:
        pass


def run(inputs, cfg=SPEC, trace=False):
    d = _derive(cfg)
    wt = _weights_from_inputs(inputs, d)
    in_maps, meta = _prep(inputs["x"], inputs["edge_index"], wt, d)
    key = (tuple(sorted(cfg.items())), meta["TT"],
           tuple(tuple(r) for r in meta["tiles"]))
    if key not in _CACHE:
        _CACHE[key] = build_module(d, meta)
    nc = _CACHE[key]
    if trace:
        _install_ntff_shim()
    res = bass_utils.run_bass_kernel_spmd(
        nc, in_maps, core_ids=list(range(d["M"])), trace=trace)
    outs = [res.results[m]["h_out"][:d["NPC_REAL"]] for m in range(d["M"])]
    full = np.concatenate(outs, axis=0).astype(np.float32)
    return full, res


def kernel(**inputs) -> np.ndarray:
    trace = bool(os.environ.get("KERNEL_TRACE"))
    full, res = run(inputs, SPEC, trace=trace)
    if trace and res.exec_time_ns is not None:
        print(f"HW exec time: {res.exec_time_ns} ns")
    return full


# revision 45
# speedup vs baseline: 1.0145x; 1.0145x over previous
"""Trainium2 Bass kernel for a 3-layer TransformerConv GNN (nn_EncoderTransformerConv).

Strategy (8 NeuronCores, SPMD, identical instruction stream per core):
  - Nodes are partitioned across cores (6250 real nodes/core, padded to 6272 =
    49 blocks of 128). Edges are partitioned by dst core.
  - Per layer:
      Phase A  : every core computes the k|v projections for ALL nodes (replicated
                 dense bf16 matmuls) and writes a row-major bf16 kv table
                 [50176, 256] to DRAM (split lo/hi so phase B can start early);
                 it also computes q (bf16, SBUF-resident) and the root-skip s for
                 its OWN nodes.
      Phase B  : per-edge work. Edges (grouped into 128-node dst blocks, split
                 into two groups by src-table half so gather indices fit int16)
                 are processed in chunks: dma_gather of kv[src] rows; q is
                 broadcast to edges with a one-hot matmul (host-precomputed S^T);
                 vector ops compute alpha = <q,k>/8, exp, ex*v; a one-hot matmul
                 (host S) performs the segmented softmax-denominator and message
                 reductions into PSUM per dst block.
      Epilogue : normalize by the segment denominator, mean heads, add skip, relu.
      Collective: AllGather of each core's h^T block (bf16) so the next layer can
                 recompute the kv table from the full h.
  - Softmax max-subtraction is skipped: alpha = q.k/8 is bounded (|alpha| < 3 for
    these weight scales); exp() is safe.
"""
import os
import sys

sys.path.insert(0, "/opt/trn_rl_repo")

import ml_dtypes
import numpy as np

import concourse.bass as bass
import concourse.bacc as bacc
import concourse.mybir as mybir
import concourse.tile as tile
from concourse import bass_utils, library_config
from concourse.masks import make_identity

F32 = mybir.dt.float32
BF16 = mybir.dt.bfloat16
I16 = mybir.dt.int16
AF = mybir.ActivationFunctionType
OP = mybir.AluOpType
BNP = ml_dtypes.bfloat16

# ----- problem dims (hardcoded per spec) -----
SPEC = dict(N=50000, E=800000, D_IN=128, HID=64, H=2, M=8)
TILE_E = 128          # edges per tile
CHUNK_T = 20          # max tiles per edge-phase chunk (whole dst-block groups)
# Trailing-pad descriptor skipping (via per-core runtime counts) measured as a
# wash: the Q7 gather ucode's time scales with scanned index slots, not emitted
# descriptors. Keep all chunks on the static full-count path.
WARM_CH = 10 ** 9     # chunks gathered with static full counts (all of them)
BLK = 128             # dst nodes per block


def _derive(cfg):
    d = dict(cfg)
    d["C"] = d["HID"]
    d["F"] = d["H"] * d["C"]            # 128 = q/k/v width
    d["WC"] = 3 * d["F"] + d["HID"]     # 448 packed q|k|v|s
    d["KV"] = 2 * d["F"]                # 256
    d["NPC_REAL"] = d["N"] // d["M"]
    d["NBLK"] = -(-d["NPC_REAL"] // BLK)
    d["NPC"] = d["NBLK"] * BLK
    d["NPAD"] = d["M"] * d["NPC"]
    d["HALF"] = d["NPAD"] // 2
    pa = 1
    for c in range(1, 9):
        if d["NBLK"] % c == 0:
            pa = c
    d["PA_CHUNK"] = pa
    d["RHSW"] = d["H"] + d["F"]         # 130 = denom cols + exv cols
    return d


def _wrap_idx(a):
    """[M, n] int -> wrapped idx layout [M, 128, n//16] (16-partition wrap,
    replicated to 8 GPSIMD core groups)."""
    Mn, n = a.shape
    w = a.reshape(Mn, n // 16, 16).transpose(0, 2, 1)
    return np.ascontiguousarray(np.tile(w, (1, 8, 1))).astype(np.int16)


def _prep(x, edge_index, weights, d):
    """Host-side preprocessing -> (in_maps, meta). weights: dict L -> (W_all[d,WC], b_all[WC])."""
    M, NPC_REAL, NPC, NPAD, HALF, NBLK = (
        d["M"], d["NPC_REAL"], d["NPC"], d["NPAD"], d["HALF"], d["NBLK"])
    N, D_IN = d["N"], d["D_IN"]

    src = np.asarray(edge_index[0]).astype(np.int64)
    dst = np.asarray(edge_index[1]).astype(np.int64)
    core = dst // NPC_REAL
    dst_l = dst - core * NPC_REAL
    blk = dst_l // BLK
    src_p = (src // NPC_REAL) * NPC + (src % NPC_REAL)
    half = (src_p >= HALF).astype(np.int64)

    counts = np.zeros((M, 2, NBLK), np.int64)
    np.add.at(counts, (core, half, blk), 1)
    tiles = np.maximum(1, -(-counts.max(axis=0) // TILE_E))  # [2, NBLK]
    flat_tiles = tiles.reshape(-1)
    tile_off = np.concatenate([[0], np.cumsum(flat_tiles)])
    TT = int(tile_off[-1])

    # stable-sort edges by (core, half, blk); rank within group
    key = (core * 2 + half) * NBLK + blk
    order = np.argsort(key, kind="stable")
    sk = key[order]
    new_run = np.ones(len(sk), bool)
    new_run[1:] = sk[1:] != sk[:-1]
    run_idx = np.cumsum(new_run) - 1
    starts = np.nonzero(new_run)[0]
    rank = np.arange(len(sk)) - starts[run_idx]
    grp = (half * NBLK + blk)[order]
    pos = tile_off[grp] * TILE_E + rank  # position within the core's edge array
    corev = core[order]

    kv_idx = np.zeros((M, TT * TILE_E), np.int64)
    dloc = np.full((M, TT * TILE_E), -1, np.int64)
    kv_idx[corev, pos] = (src_p - half * HALF)[order]
    dloc[corev, pos] = (dst_l - blk * BLK)[order]
    assert kv_idx.max() < 2 ** 15

    S = np.zeros((M, 128, TT * BLK), BNP)
    ST = np.zeros((M, 128, TT * BLK), BNP)
    dd = dloc.reshape(M, TT, TILE_E)
    mm, tt, pp = np.nonzero(dd >= 0)
    dv = dd[mm, tt, pp]
    S[mm, pp, tt * BLK + dv] = 1.0
    ST[mm, dv, tt * BLK + pp] = 1.0

    kv_w = _wrap_idx(kv_idx)

    # node features, transposed + padded (bf16)
    xT_pad = np.zeros((D_IN, NPAD), np.float32)
    n_ids = np.arange(N)
    pid = (n_ids // NPC_REAL) * NPC + (n_ids % NPC_REAL)
    xT_pad[:, pid] = np.asarray(x).T
    xT = xT_pad.astype(BNP)

    wt = {}
    for L in (1, 2, 3):
        W_all, b_all = weights[L]   # packed k|v|q|s by _weights_from_inputs
        if L == 1:
            wt["W1"] = W_all.astype(BNP)
            wt["brep1"] = np.ascontiguousarray(
                np.tile(b_all[None, :], (128, 1)).astype(np.float32))
        else:
            wt[f"W{L}"] = np.concatenate(
                [W_all, b_all[None, :]], 0).astype(BNP)

    in_maps = []
    for m in range(M):
        im = dict(
            xT=np.ascontiguousarray(xT),
            xoT=np.ascontiguousarray(xT[:, m * NPC:(m + 1) * NPC]),
            kvidx=np.ascontiguousarray(kv_w[m]),
            S_in=np.ascontiguousarray(S[m]),
            ST_in=np.ascontiguousarray(ST[m]),
            **wt,
        )
        in_maps.append(im)

    # tile metadata: (half, blk, start, stop)
    meta = []
    groups = []  # (f, b, t0, T)
    t = 0
    for f in (0, 1):
        for b in range(NBLK):
            T = int(tiles[f, b])
            groups.append((f, b, t, T))
            t += T
            for i in range(T):
                meta.append((f, b, i == 0, i == T - 1))
    # chunks = runs of whole groups (same half), <= CHUNK_T tiles, so each
    # chunk ends exactly at a group boundary (trailing pads are call-trailing)
    chunks = []
    gi = 0
    while gi < len(groups):
        f, b, t0, T = groups[gi]
        nt = T
        gj = gi + 1
        while (gj < len(groups) and groups[gj][0] == f
               and nt + groups[gj][3] <= CHUNK_T):
            nt += groups[gj][3]
            gj += 1
        chunks.append((t0, nt, f, groups[gj - 1][1]))  # last block of chunk
        gi = gj

    # per-(core, chunk) valid gather counts: for chunks beyond the warmup,
    # the final group's padding slots get idx -1 (skipped by the gather)
    nch = len(chunks)
    gcnt = np.zeros((M, nch), np.int32)
    for j, (t0, nt, f, b_last) in enumerate(chunks):
        n = nt * TILE_E
        if j < WARM_CH:
            gcnt[:, j] = n
            continue
        Tg = int(tiles[f, b_last])
        g_t0 = t0 + nt - Tg          # first tile of the final group
        for m in range(M):
            cm = int(counts[m, f, b_last])
            pad = Tg * TILE_E - cm
            gcnt[m, j] = n - pad
            if pad:
                kv_idx_w_set_m1(kv_w, m, g_t0 * TILE_E + cm, (t0 + nt) * TILE_E)
    for m in range(M):
        in_maps[m]["kvidx"] = np.ascontiguousarray(kv_w[m])
        in_maps[m]["gcnt"] = np.ascontiguousarray(gcnt[m:m + 1])
    return in_maps, dict(TT=TT, meta=meta, chunks=chunks, tiles=tiles)


def kv_idx_w_set_m1(kv_w, m, lo, hi):
    """Set wrapped-index positions [lo, hi) to -1 for core m."""
    js = np.arange(lo, hi)
    rows = (np.arange(8) * 16)[:, None] + (js % 16)[None, :]
    cols = np.broadcast_to(js // 16, rows.shape)
    kv_w[m, rows, cols] = -1


def build_module(d, meta):
    TT, chunks, tmeta = meta["TT"], meta["chunks"], meta["meta"]
    M, NPC, NPAD, HALF, NBLK, PA_CHUNK = (
        d["M"], d["NPC"], d["NPAD"], d["HALF"], d["NBLK"], d["PA_CHUNK"])
    D_IN, F, KV, WC, HID, H, C, RHSW = (
        d["D_IN"], d["F"], d["KV"], d["WC"], d["HID"], d["H"], d["C"], d["RHSW"])
    NT_ALL = NPAD // 128          # node tiles, all cores
    NCH_ALL = NT_ALL // PA_CHUNK  # phase A chunks
    RANK_CH = NBLK // PA_CHUNK    # chunks per rank slab
    CA = (3 * RANK_CH) // 4       # rank-slab chunks covered by first collective
    SPLIT_B = CA * PA_CHUNK       # blocks covered by first collective
    COLA = SPLIT_B * 128

    nc = bacc.Bacc("TRN2", target_bir_lowering=False, debug=False, num_devices=M)
    inp = {}
    for name, shape, dt in [
        ("xT", [D_IN, NPAD], BF16), ("xoT", [D_IN, NPC], BF16),
        ("W1", [D_IN, WC], BF16), ("brep1", [128, WC], F32),
        ("W2", [HID + 1, WC], BF16), ("W3", [HID + 1, WC], BF16),
        ("kvidx", [128, TT * 8], I16),
        ("gcnt", [1, len(chunks)], mybir.dt.int32),
        ("S_in", [128, TT * BLK], BF16), ("ST_in", [128, TT * BLK], BF16),
    ]:
        inp[name] = nc.dram_tensor(name, shape, dt, kind="ExternalInput")
    h_out = nc.dram_tensor("h_out", [NPC, HID], F32, kind="ExternalOutput")

    with tile.TileContext(nc) as tc:
        with tc.tile_pool(name="dram", bufs=1, space="DRAM") as dram, \
             tc.tile_pool(name="res", bufs=1) as res:
            kv_lo = dram.tile([HALF, KV], BF16)
            kv_hi = dram.tile([NPAD - HALF, KV], BF16)
            if CA > 0:
                coll_inA = dram.tile([HID + 1, COLA], BF16)
                coll_outA = dram.tile([M * (HID + 1), COLA], BF16)
            coll_inB = dram.tile([HID + 1, NPC - COLA], BF16)
            coll_outB = dram.tile([M * (HID + 1), NPC - COLA], BF16)

            nc.gpsimd.load_library(library_config.mlp)

            # resident SBUF
            W1_sb = res.tile([D_IN, WC], BF16)
            brep1_sb = res.tile([128, WC], F32)
            W2_sb = res.tile([HID + 1, WC], BF16)
            W3_sb = res.tile([HID + 1, WC], BF16)
            kvidx_sb = res.tile([128, TT * 8], I16)
            gcnt_sb = res.tile([1, len(chunks)], mybir.dt.int32)
            q_sb = res.tile([128, NBLK * F], BF16)
            s_sb = res.tile([128, NBLK * HID], F32)
            hTown = res.tile([HID + 1, NPC], BF16)
            partA = res.tile([128, NBLK * RHSW], F32)
            ident = res.tile([128, 128], F32)
            eps2 = res.tile([128, H], F32)

            for sb, t in ((W1_sb, "W1"), (brep1_sb, "brep1"), (W2_sb, "W2"),
                          (W3_sb, "W3"), (kvidx_sb, "kvidx"), (gcnt_sb, "gcnt")):
                nc.sync.dma_start(sb[:], inp[t].ap())
            make_identity(nc, ident[:])
            nc.vector.memset(hTown[HID:HID + 1, :], 1.0)
            nc.vector.memset(eps2[:], H * 1e-16)

            for layer in (1, 2, 3):
                W_sb = {1: None, 2: W2_sb, 3: W3_sb}[layer]

                # ---------- Phase A-kv: kv table for ALL nodes ----------
                with tc.tile_pool(name="pa", bufs=4) as pa, \
                     tc.tile_pool(name="pap", bufs=4, space="PSUM") as pap:

                    # own q/s first (cheap), so phase B's first blocks unblock
                    for ch in range(RANK_CH):
                        cols = slice(ch * PA_CHUNK * 128, (ch + 1) * PA_CHUNK * 128)
                        if layer == 1:
                            la = pa.tile([D_IN, PA_CHUNK * 128], BF16, tag="la")
                            nc.sync.dma_start(la[:], inp["xoT"].ap()[:, cols])
                        for t in range(PA_CHUNK):
                            gt = ch * PA_CHUNK + t  # own node tile index
                            ps = pap.tile([128, F + HID], F32, tag="psqs",
                                          name="psqs")
                            if layer == 1:
                                nc.tensor.matmul(ps[:], la[:, t * 128:(t + 1) * 128],
                                                 W1_sb[:, KV:WC],
                                                 start=True, stop=True)
                                nc.vector.tensor_tensor(
                                    q_sb[:, gt * F:(gt + 1) * F], ps[:, 0:F],
                                    brep1_sb[:, KV:KV + F], op=OP.add)
                                nc.vector.tensor_tensor(
                                    s_sb[:, gt * HID:(gt + 1) * HID],
                                    ps[:, F:F + HID],
                                    brep1_sb[:, KV + F:WC], op=OP.add)
                            else:
                                nc.tensor.matmul(
                                    ps[:],
                                    hTown[:, gt * 128:(gt + 1) * 128],
                                    W_sb[:, KV:WC], start=True, stop=True)
                                nc.scalar.copy(q_sb[:, gt * F:(gt + 1) * F],
                                               ps[:, 0:F])
                                nc.vector.tensor_copy(
                                    s_sb[:, gt * HID:(gt + 1) * HID],
                                    ps[:, F:F + HID])

                    # kv for all nodes. Order: lo table half first; within a
                    # half, chunks covered by the first (already-landed)
                    # collective before chunks needing the second one.
                    if layer == 1:
                        ch_list = list(range(NCH_ALL))
                    else:
                        ch_list = []
                        for rg in (range(0, M // 2), range(M // 2, M)):
                            for cg in (range(0, CA), range(CA, RANK_CH)):
                                for r in rg:
                                    for c in cg:
                                        ch_list.append(r * RANK_CH + c)
                    for ch in ch_list:
                        cols = slice(ch * PA_CHUNK * 128, (ch + 1) * PA_CHUNK * 128)
                        if layer == 1:
                            la = pa.tile([D_IN, PA_CHUNK * 128], BF16, tag="la")
                            nc.sync.dma_start(la[:], inp["xT"].ap()[:, cols])
                        else:
                            r, c = ch // RANK_CH, ch % RANK_CH
                            la = pa.tile([HID + 1, PA_CHUNK * 128], BF16, tag="la")
                            if c < CA:
                                src = coll_outA[r * (HID + 1):(r + 1) * (HID + 1),
                                                c * PA_CHUNK * 128:
                                                (c + 1) * PA_CHUNK * 128]
                            else:
                                src = coll_outB[r * (HID + 1):(r + 1) * (HID + 1),
                                                (c - CA) * PA_CHUNK * 128:
                                                (c - CA + 1) * PA_CHUNK * 128]
                            nc.sync.dma_start(la[:], src)
                        kvst = pa.tile([128, PA_CHUNK * KV], BF16, tag="kvst")
                        for t in range(PA_CHUNK):
                            ps = pap.tile([128, KV], F32, tag="pskv", name="pskv")
                            dstp = kvst[:, t * KV:(t + 1) * KV]
                            if layer == 1:
                                nc.tensor.matmul(ps[:], la[:, t * 128:(t + 1) * 128],
                                                 W1_sb[:, 0:KV],
                                                 start=True, stop=True)
                                nc.vector.tensor_tensor(
                                    dstp, ps[:], brep1_sb[:, 0:KV], op=OP.add)
                            else:
                                nc.tensor.matmul(ps[:], la[:, t * 128:(t + 1) * 128],
                                                 W_sb[:, 0:KV], start=True, stop=True)
                                if t % 2 == 0:
                                    nc.scalar.copy(dstp, ps[:])
                                else:
                                    nc.vector.tensor_copy(dstp, ps[:])
                        row0 = ch * PA_CHUNK * 128
                        tgt = (kv_lo[row0:row0 + PA_CHUNK * 128, :]
                               if row0 < HALF else
                               kv_hi[row0 - HALF:row0 - HALF + PA_CHUNK * 128, :])
                        nc.sync.dma_start(
                            tgt.rearrange("(t p) e -> p t e", p=128),
                            kvst[:].rearrange("p (t e) -> p t e", e=KV))

                # ---------- Phase B: edges ----------
                with tc.tile_pool(name="pb", bufs=4) as pb, \
                     tc.tile_pool(name="pb1", bufs=3) as pb1, \
                     tc.tile_pool(name="pbp", bufs=3, space="PSUM") as pbp, \
                     tc.tile_pool(name="qep", bufs=3, space="PSUM") as qep, \
                     tc.tile_pool(name="epp", bufs=1, space="PSUM") as epp, \
                     tc.tile_pool(name="ep", bufs=2) as ep:
                    psum_blk = {}
                    for j, (t0, nt, fhalf, _blast) in enumerate(chunks):
                        n = nt * TILE_E
                        nreg = n
                        kvg = pb.tile([128, CHUNK_T, KV], BF16, tag="kvg")
                        Sg = pb1.tile([128, CHUNK_T * BLK], BF16, tag="Sg")
                        STg = pb1.tile([128, CHUNK_T * BLK], BF16, tag="STg")
                        prod = pb1.tile([128, CHUNK_T * F], F32, tag="prod", bufs=2)
                        alph = pb1.tile([128, CHUNK_T * H], F32, tag="alph")
                        rhs = pb.tile([128, CHUNK_T, RHSW], BF16, tag="rhs")

                        in_ap = kv_lo[:] if fhalf == 0 else kv_hi[:]
                        nc.gpsimd.dma_gather(
                            out_ap=kvg[:, 0:nt, :], in_ap=in_ap,
                            idxs_ap=kvidx_sb[:, t0 * 8:t0 * 8 + nt * 8],
                            num_idxs=n, num_idxs_reg=nreg, elem_size=KV,
                            single_packet=False)
                        nc.sync.dma_start(
                            Sg[:, 0:n], inp["S_in"].ap()[:, t0 * BLK:t0 * BLK + n])
                        nc.sync.dma_start(
                            STg[:, 0:n], inp["ST_in"].ap()[:, t0 * BLK:t0 * BLK + n])

                        # q_edges per tile via one-hot broadcast matmul, then
                        # prod = (q_edges * 1/sqrt(C)) * k
                        for i in range(nt):
                            tg = t0 + i
                            b = tmeta[tg][1]
                            qe = qep.tile([128, F], F32, name="qe", tag="qe")
                            nc.tensor.matmul(
                                qe[:], STg[:, i * BLK:(i + 1) * BLK],
                                q_sb[:, b * F:(b + 1) * F], start=True, stop=True)
                            nc.vector.scalar_tensor_tensor(
                                out=prod[:, i * F:(i + 1) * F],
                                in0=qe[:], scalar=float(1.0 / np.sqrt(C)),
                                in1=kvg[:, i, 0:F], op0=OP.mult, op1=OP.mult)
                        # alpha[p, t, h] = sum_c prod
                        nc.vector.reduce_sum(
                            alph[:, 0:nt * H].rearrange("p (t h) -> p t h", h=H),
                            prod[:, 0:nt * F].rearrange(
                                "p (t h c) -> p t h c", h=H, c=C),
                            axis=mybir.AxisListType.X)
                        # ex = exp(alpha) -> rhs[:, :, 0:H]
                        nc.scalar.activation(
                            rhs[:, 0:nt, 0:H],
                            alph[:, 0:nt * H].rearrange("p (t h) -> p t h", h=H),
                            AF.Exp)
                        # exv = v * ex -> rhs[:, :, H:]
                        nc.vector.tensor_tensor(
                            out=rhs[:, 0:nt, H:RHSW].rearrange(
                                "p t (h c) -> p t h c", c=C),
                            in0=kvg[:, 0:nt, F:KV].rearrange(
                                "p t (h c) -> p t h c", c=C),
                            in1=rhs[:, 0:nt, 0:H].to_broadcast([128, nt, H, C]),
                            op=OP.mult)

                        for i in range(nt):
                            tg = t0 + i
                            f, b, st, sp = tmeta[tg]
                            if st:
                                psum_blk[(f, b)] = pbp.tile(
                                    [128, RHSW], F32, name="pblk", tag="pblk")
                            nc.tensor.matmul(
                                psum_blk[(f, b)][:],
                                Sg[:, i * BLK:(i + 1) * BLK],
                                rhs[:, i, :], start=st, stop=sp)
                            if not sp:
                                continue
                            ps = psum_blk.pop((f, b))
                            pa_sl = partA[:, b * RHSW:(b + 1) * RHSW]
                            if f == 0:
                                nc.scalar.copy(pa_sl, ps[:])
                                continue
                            # ---- epilogue for block b ----
                            tot = ep.tile([128, RHSW], F32, tag="tot")
                            nc.vector.tensor_tensor(tot[:], ps[:], pa_sl, op=OP.add)
                            # rec = (1/H) / (denom + 1e-16), via 1/(H*denom + H*1e-16)
                            rec = ep.tile([128, H], F32, tag="rec")
                            nc.vector.scalar_tensor_tensor(
                                out=rec[:], in0=tot[:, 0:H], scalar=float(H),
                                in1=eps2[:], op0=OP.mult, op1=OP.add)
                            nc.vector.reciprocal(rec[:], rec[:])
                            m0 = ep.tile([128, C], F32, tag="m0")
                            nc.vector.scalar_tensor_tensor(
                                out=m0[:], in0=tot[:, H:H + C],
                                scalar=rec[:, 0:1],
                                in1=s_sb[:, b * HID:(b + 1) * HID],
                                op0=OP.mult, op1=OP.add)
                            hp2 = ep.tile([128, HID], F32, tag="hp2")
                            nc.vector.scalar_tensor_tensor(
                                out=hp2[:], in0=tot[:, H + C:H + 2 * C],
                                scalar=rec[:, 1:2], in1=m0[:],
                                op0=OP.mult, op1=OP.add)
                            hblk = ep.tile([128, HID], F32, tag="hblk")
                            nc.scalar.activation(hblk[:], hp2[:], AF.Relu)
                            if layer < 3:
                                pst = epp.tile([HID, 128], F32)
                                nc.tensor.transpose(pst[:], hblk[:], ident[:])
                                nc.vector.tensor_copy(
                                    hTown[0:HID, b * 128:(b + 1) * 128], pst[:])
                                if CA > 0 and b == SPLIT_B - 1:
                                    # first half of h^T is final: allgather it
                                    # while the rest of phase B runs
                                    nc.sync.dma_start(coll_inA[:, :],
                                                      hTown[:, 0:COLA])
                                    nc.gpsimd.collective_compute(
                                        "AllGather", OP.bypass,
                                        ins=[coll_inA.opt()],
                                        outs=[coll_outA.opt()],
                                        replica_groups=[list(range(M))])
                            else:
                                nc.sync.dma_start(
                                    h_out.ap()[b * 128:(b + 1) * 128, :], hblk[:])
                    assert not psum_blk

                if layer < 3:
                    nc.sync.dma_start(coll_inB[:, :], hTown[:, COLA:])
                    nc.gpsimd.collective_compute(
                        "AllGather", OP.bypass,
                        ins=[coll_inB.opt()], outs=[coll_outB.opt()],
                        replica_groups=[list(range(M))])
    nc.compile()
    return nc


# ---------------- public entry ----------------
_CACHE = {}


def _weights_from_inputs(inputs, d):
    # packed column order: k | v | q | s
    wt = {}
    for L in (1, 2, 3):
        W_all = np.concatenate(
            [np.asarray(inputs[f"W{L}{nm}"], np.float32) for nm in ("k", "v", "q", "s")],
            axis=1)
        b_all = np.concatenate(
            [np.asarray(inputs[f"b{L}{nm}"], np.float32) for nm in ("k", "v", "q", "s")])
        wt[L] = (W_all, b_all)
    return wt


def _install_ntff_shim():
    import types
    if "antenv.axon_hooks" in sys.modules:
        return
    try:
        from trn_agent_boot.trn_boot import _ntff_profile_via_ctypes
        hook = _ntff_profile_via_ctypes("/opt/axon/libaxon_pjrt.so")
    except Exception:
        hook = None
    mod = types.ModuleType("antenv.axon_hooks")
    mod.get_axon_ntff_profile_hook = lambda: hook
    mod.set_axon_ntff_profile_hook = lambda h: None
    sys.modules["antenv.axon_hooks"] = mod
    try:
        import antenv
        antenv.axon_hooks = mod
    except Exception:
        pass


def run(inputs, cfg=SPEC, trace=False):
    d = _derive(cfg)
    wt = _weights_from_inputs(inputs, d)
    in_maps, meta = _prep(inputs["x"], inputs["edge_index"], wt, d)
    key = (tuple(sorted(cfg.items())), meta["TT"],
           tuple(tuple(r) for r in meta["tiles"]))
    if key not in _CACHE:
        _CACHE[key] = build_module(d, meta)
    nc = _CACHE[key]
    if trace:
        _install_ntff_shim()
    res = bass_utils.run_bass_kernel_spmd(
        nc, in_maps, core_ids=list(range(d["M"])), trace=trace)
    outs = [res.results[m]["h_out"][:d["NPC_REAL"]] for m in range(d["M"])]
    full = np.concatenate(outs, axis=0).astype(np.float32)
    return full, res


def kernel(**inputs) -> np.ndarray:
    trace = bool(os.environ.get("KERNEL_TRACE"))
    full, res = run(inputs, SPEC, trace=trace)
    if trace and res.exec_time_ns is not None:
        print(f"HW exec time: {res.exec_time_ns} ns")
    return full


# revision 47
# speedup vs baseline: 1.0202x; 1.0056x over previous
"""Trainium2 Bass kernel for a 3-layer TransformerConv GNN (nn_EncoderTransformerConv).

Strategy (8 NeuronCores, SPMD, identical instruction stream per core):
  - Nodes are partitioned across cores (6250 real nodes/core, padded to 6272 =
    49 blocks of 128). Edges are partitioned by dst core.
  - Per layer:
      Phase A  : every core computes the k|v projections for ALL nodes (replicated
                 dense bf16 matmuls) and writes a row-major bf16 kv table
                 [50176, 256] to DRAM (split lo/hi so phase B can start early);
                 it also computes q (bf16, SBUF-resident) and the root-skip s for
                 its OWN nodes.
      Phase B  : per-edge work. Edges (grouped into 128-node dst blocks, split
                 into two groups by src-table half so gather indices fit int16)
                 are processed in chunks: dma_gather of kv[src] rows; q is
                 broadcast to edges with a one-hot matmul (host-precomputed S^T);
                 vector ops compute alpha = <q,k>/8, exp, ex*v; a one-hot matmul
                 (host S) performs the segmented softmax-denominator and message
                 reductions into PSUM per dst block.
      Epilogue : normalize by the segment denominator, mean heads, add skip, relu.
      Collective: AllGather of each core's h^T block (bf16) so the next layer can
                 recompute the kv table from the full h.
  - Softmax max-subtraction is skipped: alpha = q.k/8 is bounded (|alpha| < 3 for
    these weight scales); exp() is safe.
"""
import os
import sys

sys.path.insert(0, "/opt/trn_rl_repo")

import ml_dtypes
import numpy as np

import concourse.bass as bass
import concourse.bacc as bacc
import concourse.mybir as mybir
import concourse.tile as tile
from concourse import bass_utils, library_config
from concourse.masks import make_identity

F32 = mybir.dt.float32
BF16 = mybir.dt.bfloat16
I16 = mybir.dt.int16
AF = mybir.ActivationFunctionType
OP = mybir.AluOpType
BNP = ml_dtypes.bfloat16

# ----- problem dims (hardcoded per spec) -----
SPEC = dict(N=50000, E=800000, D_IN=128, HID=64, H=2, M=8)
TILE_E = 128          # edges per tile
CHUNK_T = 20          # max tiles per edge-phase chunk (whole dst-block groups)
# Trailing-pad descriptor skipping (via per-core runtime counts) measured as a
# wash: the Q7 gather ucode's time scales with scanned index slots, not emitted
# descriptors. Keep all chunks on the static full-count path.
WARM_CH = 10 ** 9     # chunks gathered with static full counts (all of them)
BLK = 128             # dst nodes per block


def _derive(cfg):
    d = dict(cfg)
    d["C"] = d["HID"]
    d["F"] = d["H"] * d["C"]            # 128 = q/k/v width
    d["WC"] = 3 * d["F"] + d["HID"]     # 448 packed q|k|v|s
    d["KV"] = 2 * d["F"]                # 256
    d["NPC_REAL"] = d["N"] // d["M"]
    d["NBLK"] = -(-d["NPC_REAL"] // BLK)
    d["NPC"] = d["NBLK"] * BLK
    d["NPAD"] = d["M"] * d["NPC"]
    d["HALF"] = d["NPAD"] // 2
    pa = 1
    for c in range(1, 9):
        if d["NBLK"] % c == 0:
            pa = c
    d["PA_CHUNK"] = pa
    d["RHSW"] = d["H"] + d["F"]         # 130 = denom cols + exv cols
    return d


def _wrap_idx(a):
    """[M, n] int -> wrapped idx layout [M, 128, n//16] (16-partition wrap,
    replicated to 8 GPSIMD core groups)."""
    Mn, n = a.shape
    w = a.reshape(Mn, n // 16, 16).transpose(0, 2, 1)
    return np.ascontiguousarray(np.tile(w, (1, 8, 1))).astype(np.int16)


def _prep(x, edge_index, weights, d):
    """Host-side preprocessing -> (in_maps, meta). weights: dict L -> (W_all[d,WC], b_all[WC])."""
    M, NPC_REAL, NPC, NPAD, HALF, NBLK = (
        d["M"], d["NPC_REAL"], d["NPC"], d["NPAD"], d["HALF"], d["NBLK"])
    N, D_IN = d["N"], d["D_IN"]

    src = np.asarray(edge_index[0]).astype(np.int64)
    dst = np.asarray(edge_index[1]).astype(np.int64)
    core = dst // NPC_REAL
    dst_l = dst - core * NPC_REAL
    blk = dst_l // BLK
    src_p = (src // NPC_REAL) * NPC + (src % NPC_REAL)
    half = (src_p >= HALF).astype(np.int64)

    counts = np.zeros((M, 2, NBLK), np.int64)
    np.add.at(counts, (core, half, blk), 1)
    tiles = np.maximum(1, -(-counts.max(axis=0) // TILE_E))  # [2, NBLK]
    flat_tiles = tiles.reshape(-1)
    tile_off = np.concatenate([[0], np.cumsum(flat_tiles)])
    TT = int(tile_off[-1])

    # stable-sort edges by (core, half, blk); rank within group
    key = (core * 2 + half) * NBLK + blk
    order = np.argsort(key, kind="stable")
    sk = key[order]
    new_run = np.ones(len(sk), bool)
    new_run[1:] = sk[1:] != sk[:-1]
    run_idx = np.cumsum(new_run) - 1
    starts = np.nonzero(new_run)[0]
    rank = np.arange(len(sk)) - starts[run_idx]
    grp = (half * NBLK + blk)[order]
    pos = tile_off[grp] * TILE_E + rank  # position within the core's edge array
    corev = core[order]

    kv_idx = np.zeros((M, TT * TILE_E), np.int64)
    dloc = np.full((M, TT * TILE_E), -1, np.int64)
    kv_idx[corev, pos] = (src_p - half * HALF)[order]
    dloc[corev, pos] = (dst_l - blk * BLK)[order]
    assert kv_idx.max() < 2 ** 15

    S = np.zeros((M, 128, TT * BLK), BNP)
    ST = np.zeros((M, 128, TT * BLK), BNP)
    dd = dloc.reshape(M, TT, TILE_E)
    mm, tt, pp = np.nonzero(dd >= 0)
    dv = dd[mm, tt, pp]
    S[mm, pp, tt * BLK + dv] = 1.0
    ST[mm, dv, tt * BLK + pp] = 1.0

    kv_w = _wrap_idx(kv_idx)

    # node features, transposed + padded (bf16)
    xT_pad = np.zeros((D_IN, NPAD), np.float32)
    n_ids = np.arange(N)
    pid = (n_ids // NPC_REAL) * NPC + (n_ids % NPC_REAL)
    xT_pad[:, pid] = np.asarray(x).T
    xT = xT_pad.astype(BNP)

    wt = {}
    for L in (1, 2, 3):
        W_all, b_all = weights[L]   # packed k|v|q|s by _weights_from_inputs
        if L == 1:
            wt["W1"] = W_all.astype(BNP)
            wt["brep1"] = np.ascontiguousarray(
                np.tile(b_all[None, :], (128, 1)).astype(np.float32))
        else:
            wt[f"W{L}"] = np.concatenate(
                [W_all, b_all[None, :]], 0).astype(BNP)

    in_maps = []
    for m in range(M):
        im = dict(
            xT=np.ascontiguousarray(xT),
            xoT=np.ascontiguousarray(xT[:, m * NPC:(m + 1) * NPC]),
            kvidx=np.ascontiguousarray(kv_w[m]),
            S_in=np.ascontiguousarray(S[m]),
            ST_in=np.ascontiguousarray(ST[m]),
            **wt,
        )
        in_maps.append(im)

    # tile metadata: (half, blk, start, stop)
    meta = []
    groups = []  # (f, b, t0, T)
    t = 0
    for f in (0, 1):
        for b in range(NBLK):
            T = int(tiles[f, b])
            groups.append((f, b, t, T))
            t += T
            for i in range(T):
                meta.append((f, b, i == 0, i == T - 1))
    # chunks = runs of whole groups (same half), <= CHUNK_T tiles, so each
    # chunk ends exactly at a group boundary (trailing pads are call-trailing)
    chunks = []
    gi = 0
    while gi < len(groups):
        f, b, t0, T = groups[gi]
        nt = T
        gj = gi + 1
        while (gj < len(groups) and groups[gj][0] == f
               and nt + groups[gj][3] <= CHUNK_T):
            nt += groups[gj][3]
            gj += 1
        chunks.append((t0, nt, f, groups[gj - 1][1]))  # last block of chunk
        gi = gj

    # per-(core, chunk) valid gather counts: for chunks beyond the warmup,
    # the final group's padding slots get idx -1 (skipped by the gather)
    nch = len(chunks)
    gcnt = np.zeros((M, nch), np.int32)
    for j, (t0, nt, f, b_last) in enumerate(chunks):
        n = nt * TILE_E
        if j < WARM_CH:
            gcnt[:, j] = n
            continue
        Tg = int(tiles[f, b_last])
        g_t0 = t0 + nt - Tg          # first tile of the final group
        for m in range(M):
            cm = int(counts[m, f, b_last])
            pad = Tg * TILE_E - cm
            gcnt[m, j] = n - pad
            if pad:
                kv_idx_w_set_m1(kv_w, m, g_t0 * TILE_E + cm, (t0 + nt) * TILE_E)
    for m in range(M):
        in_maps[m]["kvidx"] = np.ascontiguousarray(kv_w[m])
        in_maps[m]["gcnt"] = np.ascontiguousarray(gcnt[m:m + 1])
    return in_maps, dict(TT=TT, meta=meta, chunks=chunks, tiles=tiles)


def kv_idx_w_set_m1(kv_w, m, lo, hi):
    """Set wrapped-index positions [lo, hi) to -1 for core m."""
    js = np.arange(lo, hi)
    rows = (np.arange(8) * 16)[:, None] + (js % 16)[None, :]
    cols = np.broadcast_to(js // 16, rows.shape)
    kv_w[m, rows, cols] = -1


def build_module(d, meta):
    TT, chunks, tmeta = meta["TT"], meta["chunks"], meta["meta"]
    M, NPC, NPAD, HALF, NBLK, PA_CHUNK = (
        d["M"], d["NPC"], d["NPAD"], d["HALF"], d["NBLK"], d["PA_CHUNK"])
    D_IN, F, KV, WC, HID, H, C, RHSW = (
        d["D_IN"], d["F"], d["KV"], d["WC"], d["HID"], d["H"], d["C"], d["RHSW"])
    NT_ALL = NPAD // 128          # node tiles, all cores
    NCH_ALL = NT_ALL // PA_CHUNK  # phase A chunks
    RANK_CH = NBLK // PA_CHUNK    # chunks per rank slab
    CA = max(0, RANK_CH - 1)      # rank-slab chunks covered by first collective
    SPLIT_B = CA * PA_CHUNK       # blocks covered by first collective
    COLA = SPLIT_B * 128

    nc = bacc.Bacc("TRN2", target_bir_lowering=False, debug=False, num_devices=M)
    inp = {}
    for name, shape, dt in [
        ("xT", [D_IN, NPAD], BF16), ("xoT", [D_IN, NPC], BF16),
        ("W1", [D_IN, WC], BF16), ("brep1", [128, WC], F32),
        ("W2", [HID + 1, WC], BF16), ("W3", [HID + 1, WC], BF16),
        ("kvidx", [128, TT * 8], I16),
        ("gcnt", [1, len(chunks)], mybir.dt.int32),
        ("S_in", [128, TT * BLK], BF16), ("ST_in", [128, TT * BLK], BF16),
    ]:
        inp[name] = nc.dram_tensor(name, shape, dt, kind="ExternalInput")
    h_out = nc.dram_tensor("h_out", [NPC, HID], F32, kind="ExternalOutput")

    with tile.TileContext(nc) as tc:
        with tc.tile_pool(name="dram", bufs=1, space="DRAM") as dram, \
             tc.tile_pool(name="res", bufs=1) as res:
            kv_lo = dram.tile([HALF, KV], BF16)
            kv_hi = dram.tile([NPAD - HALF, KV], BF16)
            if CA > 0:
                coll_inA = dram.tile([HID + 1, COLA], BF16)
                coll_outA = dram.tile([M * (HID + 1), COLA], BF16)
            coll_inB = dram.tile([HID + 1, NPC - COLA], BF16)
            coll_outB = dram.tile([M * (HID + 1), NPC - COLA], BF16)

            nc.gpsimd.load_library(library_config.mlp)

            # resident SBUF
            W1_sb = res.tile([D_IN, WC], BF16)
            brep1_sb = res.tile([128, WC], F32)
            W2_sb = res.tile([HID + 1, WC], BF16)
            W3_sb = res.tile([HID + 1, WC], BF16)
            kvidx_sb = res.tile([128, TT * 8], I16)
            gcnt_sb = res.tile([1, len(chunks)], mybir.dt.int32)
            q_sb = res.tile([128, NBLK * F], BF16)
            s_sb = res.tile([128, NBLK * HID], F32)
            hTown = res.tile([HID + 1, NPC], BF16)
            partA = res.tile([128, NBLK * RHSW], F32)
            ident = res.tile([128, 128], F32)
            eps2 = res.tile([128, H], F32)

            for sb, t in ((W1_sb, "W1"), (brep1_sb, "brep1"), (W2_sb, "W2"),
                          (W3_sb, "W3"), (kvidx_sb, "kvidx"), (gcnt_sb, "gcnt")):
                nc.sync.dma_start(sb[:], inp[t].ap())
            make_identity(nc, ident[:])
            nc.vector.memset(hTown[HID:HID + 1, :], 1.0)
            nc.vector.memset(eps2[:], H * 1e-16)

            for layer in (1, 2, 3):
                W_sb = {1: None, 2: W2_sb, 3: W3_sb}[layer]

                # ---------- Phase A-kv: kv table for ALL nodes ----------
                with tc.tile_pool(name="pa", bufs=4) as pa, \
                     tc.tile_pool(name="pap", bufs=4, space="PSUM") as pap:

                    # own q/s first (cheap), so phase B's first blocks unblock
                    for ch in range(RANK_CH):
                        cols = slice(ch * PA_CHUNK * 128, (ch + 1) * PA_CHUNK * 128)
                        if layer == 1:
                            la = pa.tile([D_IN, PA_CHUNK * 128], BF16, tag="la")
                            nc.sync.dma_start(la[:], inp["xoT"].ap()[:, cols])
                        for t in range(PA_CHUNK):
                            gt = ch * PA_CHUNK + t  # own node tile index
                            ps = pap.tile([128, F + HID], F32, tag="psqs",
                                          name="psqs")
                            if layer == 1:
                                nc.tensor.matmul(ps[:], la[:, t * 128:(t + 1) * 128],
                                                 W1_sb[:, KV:WC],
                                                 start=True, stop=True)
                                nc.vector.tensor_tensor(
                                    q_sb[:, gt * F:(gt + 1) * F], ps[:, 0:F],
                                    brep1_sb[:, KV:KV + F], op=OP.add)
                                nc.vector.tensor_tensor(
                                    s_sb[:, gt * HID:(gt + 1) * HID],
                                    ps[:, F:F + HID],
                                    brep1_sb[:, KV + F:WC], op=OP.add)
                            else:
                                nc.tensor.matmul(
                                    ps[:],
                                    hTown[:, gt * 128:(gt + 1) * 128],
                                    W_sb[:, KV:WC], start=True, stop=True)
                                nc.scalar.copy(q_sb[:, gt * F:(gt + 1) * F],
                                               ps[:, 0:F])
                                nc.vector.tensor_copy(
                                    s_sb[:, gt * HID:(gt + 1) * HID],
                                    ps[:, F:F + HID])

                    # kv for all nodes. Order: lo table half first; within a
                    # half, chunks covered by the first (already-landed)
                    # collective before chunks needing the second one.
                    if layer == 1:
                        ch_list = list(range(NCH_ALL))
                    else:
                        ch_list = []
                        for rg in (range(0, M // 2), range(M // 2, M)):
                            for cg in (range(0, CA), range(CA, RANK_CH)):
                                for r in rg:
                                    for c in cg:
                                        ch_list.append(r * RANK_CH + c)
                    for ch in ch_list:
                        cols = slice(ch * PA_CHUNK * 128, (ch + 1) * PA_CHUNK * 128)
                        if layer == 1:
                            la = pa.tile([D_IN, PA_CHUNK * 128], BF16, tag="la")
                            nc.sync.dma_start(la[:], inp["xT"].ap()[:, cols])
                        else:
                            r, c = ch // RANK_CH, ch % RANK_CH
                            la = pa.tile([HID + 1, PA_CHUNK * 128], BF16, tag="la")
                            if c < CA:
                                src = coll_outA[r * (HID + 1):(r + 1) * (HID + 1),
                                                c * PA_CHUNK * 128:
                                                (c + 1) * PA_CHUNK * 128]
                            else:
                                src = coll_outB[r * (HID + 1):(r + 1) * (HID + 1),
                                                (c - CA) * PA_CHUNK * 128:
                                                (c - CA + 1) * PA_CHUNK * 128]
                            nc.sync.dma_start(la[:], src)
                        kvst = pa.tile([128, PA_CHUNK * KV], BF16, tag="kvst")
                        for t in range(PA_CHUNK):
                            ps = pap.tile([128, KV], F32, tag="pskv", name="pskv")
                            dstp = kvst[:, t * KV:(t + 1) * KV]
                            if layer == 1:
                                nc.tensor.matmul(ps[:], la[:, t * 128:(t + 1) * 128],
                                                 W1_sb[:, 0:KV],
                                                 start=True, stop=True)
                                nc.vector.tensor_tensor(
                                    dstp, ps[:], brep1_sb[:, 0:KV], op=OP.add)
                            else:
                                nc.tensor.matmul(ps[:], la[:, t * 128:(t + 1) * 128],
                                                 W_sb[:, 0:KV], start=True, stop=True)
                                if t % 2 == 0:
                                    nc.scalar.copy(dstp, ps[:])
                                else:
                                    nc.vector.tensor_copy(dstp, ps[:])
                        row0 = ch * PA_CHUNK * 128
                        tgt = (kv_lo[row0:row0 + PA_CHUNK * 128, :]
                               if row0 < HALF else
                               kv_hi[row0 - HALF:row0 - HALF + PA_CHUNK * 128, :])
                        nc.sync.dma_start(
                            tgt.rearrange("(t p) e -> p t e", p=128),
                            kvst[:].rearrange("p (t e) -> p t e", e=KV))

                # ---------- Phase B: edges ----------
                with tc.tile_pool(name="pb", bufs=4) as pb, \
                     tc.tile_pool(name="pb1", bufs=3) as pb1, \
                     tc.tile_pool(name="pbp", bufs=3, space="PSUM") as pbp, \
                     tc.tile_pool(name="qep", bufs=4, space="PSUM") as qep, \
                     tc.tile_pool(name="epp", bufs=1, space="PSUM") as epp, \
                     tc.tile_pool(name="ep", bufs=2) as ep:
                    psum_blk = {}
                    for j, (t0, nt, fhalf, _blast) in enumerate(chunks):
                        n = nt * TILE_E
                        nreg = n
                        kvg = pb.tile([128, CHUNK_T, KV], BF16, tag="kvg")
                        Sg = pb1.tile([128, CHUNK_T * BLK], BF16, tag="Sg")
                        STg = pb1.tile([128, CHUNK_T * BLK], BF16, tag="STg")
                        prod = pb1.tile([128, CHUNK_T * F], F32, tag="prod", bufs=2)
                        alph = pb1.tile([128, CHUNK_T * H], F32, tag="alph")
                        rhs = pb.tile([128, CHUNK_T, RHSW], BF16, tag="rhs")

                        in_ap = kv_lo[:] if fhalf == 0 else kv_hi[:]
                        nc.gpsimd.dma_gather(
                            out_ap=kvg[:, 0:nt, :], in_ap=in_ap,
                            idxs_ap=kvidx_sb[:, t0 * 8:t0 * 8 + nt * 8],
                            num_idxs=n, num_idxs_reg=nreg, elem_size=KV,
                            single_packet=False)
                        nc.sync.dma_start(
                            Sg[:, 0:n], inp["S_in"].ap()[:, t0 * BLK:t0 * BLK + n])
                        nc.sync.dma_start(
                            STg[:, 0:n], inp["ST_in"].ap()[:, t0 * BLK:t0 * BLK + n])

                        # q_edges per tile via one-hot broadcast matmul, then
                        # prod = (q_edges * 1/sqrt(C)) * k
                        for i in range(nt):
                            tg = t0 + i
                            b = tmeta[tg][1]
                            qe = qep.tile([128, F], F32, name="qe", tag="qe")
                            nc.tensor.matmul(
                                qe[:], STg[:, i * BLK:(i + 1) * BLK],
                                q_sb[:, b * F:(b + 1) * F], start=True, stop=True)
                            nc.vector.scalar_tensor_tensor(
                                out=prod[:, i * F:(i + 1) * F],
                                in0=qe[:], scalar=float(1.0 / np.sqrt(C)),
                                in1=kvg[:, i, 0:F], op0=OP.mult, op1=OP.mult)
                        # alpha[p, t, h] = sum_c prod
                        nc.vector.reduce_sum(
                            alph[:, 0:nt * H].rearrange("p (t h) -> p t h", h=H),
                            prod[:, 0:nt * F].rearrange(
                                "p (t h c) -> p t h c", h=H, c=C),
                            axis=mybir.AxisListType.X)
                        # ex = exp(alpha) -> rhs[:, :, 0:H]
                        nc.scalar.activation(
                            rhs[:, 0:nt, 0:H],
                            alph[:, 0:nt * H].rearrange("p (t h) -> p t h", h=H),
                            AF.Exp)
                        # exv = v * ex -> rhs[:, :, H:]
                        nc.vector.tensor_tensor(
                            out=rhs[:, 0:nt, H:RHSW].rearrange(
                                "p t (h c) -> p t h c", c=C),
                            in0=kvg[:, 0:nt, F:KV].rearrange(
                                "p t (h c) -> p t h c", c=C),
                            in1=rhs[:, 0:nt, 0:H].to_broadcast([128, nt, H, C]),
                            op=OP.mult)

                        for i in range(nt):
                            tg = t0 + i
                            f, b, st, sp = tmeta[tg]
                            if st:
                                psum_blk[(f, b)] = pbp.tile(
                                    [128, RHSW], F32, name="pblk", tag="pblk")
                            nc.tensor.matmul(
                                psum_blk[(f, b)][:],
                                Sg[:, i * BLK:(i + 1) * BLK],
                                rhs[:, i, :], start=st, stop=sp)
                            if not sp:
                                continue
                            ps = psum_blk.pop((f, b))
                            pa_sl = partA[:, b * RHSW:(b + 1) * RHSW]
                            if f == 0:
                                nc.scalar.copy(pa_sl, ps[:])
                                continue
                            # ---- epilogue for block b ----
                            tot = ep.tile([128, RHSW], F32, tag="tot")
                            nc.vector.tensor_tensor(tot[:], ps[:], pa_sl, op=OP.add)
                            # rec = (1/H) / (denom + 1e-16), via 1/(H*denom + H*1e-16)
                            rec = ep.tile([128, H], F32, tag="rec")
                            nc.vector.scalar_tensor_tensor(
                                out=rec[:], in0=tot[:, 0:H], scalar=float(H),
                                in1=eps2[:], op0=OP.mult, op1=OP.add)
                            nc.vector.reciprocal(rec[:], rec[:])
                            m0 = ep.tile([128, C], F32, tag="m0")
                            nc.vector.scalar_tensor_tensor(
                                out=m0[:], in0=tot[:, H:H + C],
                                scalar=rec[:, 0:1],
                                in1=s_sb[:, b * HID:(b + 1) * HID],
                                op0=OP.mult, op1=OP.add)
                            hp2 = ep.tile([128, HID], F32, tag="hp2")
                            nc.vector.scalar_tensor_tensor(
                                out=hp2[:], in0=tot[:, H + C:H + 2 * C],
                                scalar=rec[:, 1:2], in1=m0[:],
                                op0=OP.mult, op1=OP.add)
                            hblk = ep.tile([128, HID], F32, tag="hblk")
                            nc.scalar.activation(hblk[:], hp2[:], AF.Relu)
                            if layer < 3:
                                pst = epp.tile([HID, 128], F32)
                                nc.tensor.transpose(pst[:], hblk[:], ident[:])
                                nc.vector.tensor_copy(
                                    hTown[0:HID, b * 128:(b + 1) * 128], pst[:])
                                if CA > 0 and b == SPLIT_B - 1:
                                    # first half of h^T is final: allgather it
                                    # while the rest of phase B runs
                                    nc.sync.dma_start(coll_inA[:, :],
                                                      hTown[:, 0:COLA])
                                    nc.gpsimd.collective_compute(
                                        "AllGather", OP.bypass,
                                        ins=[coll_inA.opt()],
                                        outs=[coll_outA.opt()],
                                        replica_groups=[list(range(M))])
                            else:
                                nc.sync.dma_start(
                                    h_out.ap()[b * 128:(b + 1) * 128, :], hblk[:])
                    assert not psum_blk

                if layer < 3:
                    nc.sync.dma_start(coll_inB[:, :], hTown[:, COLA:])
                    nc.gpsimd.collective_compute(
                        "AllGather", OP.bypass,
                        ins=[coll_inB.opt()], outs=[coll_outB.opt()],
                        replica_groups=[list(range(M))])
    nc.compile()
    return nc


# ---------------- public entry ----------------
_CACHE = {}


def _weights_from_inputs(inputs, d):
    # packed column order: k | v | q | s
    wt = {}
    for L in (1, 2, 3):
        W_all = np.concatenate(
            [np.asarray(inputs[f"W{L}{nm}"], np.float32) for nm in ("k", "v", "q", "s")],
            axis=1)
        b_all = np.concatenate(
            [np.asarray(inputs[f"b{L}{nm}"], np.float32) for nm in ("k", "v", "q", "s")])
        wt[L] = (W_all, b_all)
    return wt


def _install_ntff_shim():
    import types
    if "antenv.axon_hooks" in sys.modules:
        return
    try:
        from trn_agent_boot.trn_boot import _ntff_profile_via_ctypes
        hook = _ntff_profile_via_ctypes("/opt/axon/libaxon_pjrt.so")
    except Exception:
        hook = None
    mod = types.ModuleType("antenv.axon_hooks")
    mod.get_axon_ntff_profile_hook = lambda: hook
    mod.set_axon_ntff_profile_hook = lambda h: None
    sys.modules["antenv.axon_hooks"] = mod
    try:
        import antenv
        antenv.axon_hooks = mod
    except Exception:
        pass


def run(inputs, cfg=SPEC, trace=False):
    d = _derive(cfg)
    wt = _weights_from_inputs(inputs, d)
    in_maps, meta = _prep(inputs["x"], inputs["edge_index"], wt, d)
    key = (tuple(sorted(cfg.items())), meta["TT"],
           tuple(tuple(r) for r in meta["tiles"]))
    if key not in _CACHE:
        _CACHE[key] = build_module(d, meta)
    nc = _CACHE[key]
    if trace:
        _install_ntff_shim()
    res = bass_utils.run_bass_kernel_spmd(
        nc, in_maps, core_ids=list(range(d["M"])), trace=trace)
    outs = [res.results[m]["h_out"][:d["NPC_REAL"]] for m in range(d["M"])]
    full = np.concatenate(outs, axis=0).astype(np.float32)
    return full, res


def kernel(**inputs) -> np.ndarray:
    trace = bool(os.environ.get("KERNEL_TRACE"))
    full, res = run(inputs, SPEC, trace=trace)
    if trace and res.exec_time_ns is not None:
        print(f"HW exec time: {res.exec_time_ns} ns")
    return full


# revision 48
# speedup vs baseline: 1.0203x; 1.0002x over previous
"""Trainium2 Bass kernel for a 3-layer TransformerConv GNN (nn_EncoderTransformerConv).

Strategy (8 NeuronCores, SPMD, identical instruction stream per core):
  - Nodes are partitioned across cores (6250 real nodes/core, padded to 6272 =
    49 blocks of 128). Edges are partitioned by dst core.
  - Per layer:
      Phase A  : every core computes the k|v projections for ALL nodes (replicated
                 dense bf16 matmuls) and writes a row-major bf16 kv table
                 [50176, 256] to DRAM (split lo/hi so phase B can start early);
                 it also computes q (bf16, SBUF-resident) and the root-skip s for
                 its OWN nodes.
      Phase B  : per-edge work. Edges (grouped into 128-node dst blocks, split
                 into two groups by src-table half so gather indices fit int16)
                 are processed in chunks: dma_gather of kv[src] rows; q is
                 broadcast to edges with a one-hot matmul (host-precomputed S^T);
                 vector ops compute alpha = <q,k>/8, exp, ex*v; a one-hot matmul
                 (host S) performs the segmented softmax-denominator and message
                 reductions into PSUM per dst block.
      Epilogue : normalize by the segment denominator, mean heads, add skip, relu.
      Collective: AllGather of each core's h^T block (bf16) so the next layer can
                 recompute the kv table from the full h.
  - Softmax max-subtraction is skipped: alpha = q.k/8 is bounded (|alpha| < 3 for
    these weight scales); exp() is safe.
"""
import os
import sys

sys.path.insert(0, "/opt/trn_rl_repo")

import ml_dtypes
import numpy as np

import concourse.bass as bass
import concourse.bacc as bacc
import concourse.mybir as mybir
import concourse.tile as tile
from concourse import bass_utils, library_config
from concourse.masks import make_identity

F32 = mybir.dt.float32
BF16 = mybir.dt.bfloat16
I16 = mybir.dt.int16
AF = mybir.ActivationFunctionType
OP = mybir.AluOpType
BNP = ml_dtypes.bfloat16

# ----- problem dims (hardcoded per spec) -----
SPEC = dict(N=50000, E=800000, D_IN=128, HID=64, H=2, M=8)
TILE_E = 128          # edges per tile
CHUNK_T = 20          # max tiles per edge-phase chunk (whole dst-block groups)
# Trailing-pad descriptor skipping (via per-core runtime counts) measured as a
# wash: the Q7 gather ucode's time scales with scanned index slots, not emitted
# descriptors. Keep all chunks on the static full-count path.
WARM_CH = 10 ** 9     # chunks gathered with static full counts (all of them)
BLK = 128             # dst nodes per block


def _derive(cfg):
    d = dict(cfg)
    d["C"] = d["HID"]
    d["F"] = d["H"] * d["C"]            # 128 = q/k/v width
    d["WC"] = 3 * d["F"] + d["HID"]     # 448 packed q|k|v|s
    d["KV"] = 2 * d["F"]                # 256
    d["NPC_REAL"] = d["N"] // d["M"]
    d["NBLK"] = -(-d["NPC_REAL"] // BLK)
    d["NPC"] = d["NBLK"] * BLK
    d["NPAD"] = d["M"] * d["NPC"]
    d["HALF"] = d["NPAD"] // 2
    pa = 1
    for c in range(1, 9):
        if d["NBLK"] % c == 0:
            pa = c
    d["PA_CHUNK"] = pa
    d["RHSW"] = d["H"] + d["F"]         # 130 = denom cols + exv cols
    return d


def _wrap_idx(a):
    """[M, n] int -> wrapped idx layout [M, 128, n//16] (16-partition wrap,
    replicated to 8 GPSIMD core groups)."""
    Mn, n = a.shape
    w = a.reshape(Mn, n // 16, 16).transpose(0, 2, 1)
    return np.ascontiguousarray(np.tile(w, (1, 8, 1))).astype(np.int16)


def _prep(x, edge_index, weights, d):
    """Host-side preprocessing -> (in_maps, meta). weights: dict L -> (W_all[d,WC], b_all[WC])."""
    M, NPC_REAL, NPC, NPAD, HALF, NBLK = (
        d["M"], d["NPC_REAL"], d["NPC"], d["NPAD"], d["HALF"], d["NBLK"])
    N, D_IN = d["N"], d["D_IN"]

    src = np.asarray(edge_index[0]).astype(np.int64)
    dst = np.asarray(edge_index[1]).astype(np.int64)
    core = dst // NPC_REAL
    dst_l = dst - core * NPC_REAL
    blk = dst_l // BLK
    src_p = (src // NPC_REAL) * NPC + (src % NPC_REAL)
    half = (src_p >= HALF).astype(np.int64)

    counts = np.zeros((M, 2, NBLK), np.int64)
    np.add.at(counts, (core, half, blk), 1)
    tiles = np.maximum(1, -(-counts.max(axis=0) // TILE_E))  # [2, NBLK]
    flat_tiles = tiles.reshape(-1)
    tile_off = np.concatenate([[0], np.cumsum(flat_tiles)])
    TT = int(tile_off[-1])

    # stable-sort edges by (core, half, blk); rank within group
    key = (core * 2 + half) * NBLK + blk
    order = np.argsort(key, kind="stable")
    sk = key[order]
    new_run = np.ones(len(sk), bool)
    new_run[1:] = sk[1:] != sk[:-1]
    run_idx = np.cumsum(new_run) - 1
    starts = np.nonzero(new_run)[0]
    rank = np.arange(len(sk)) - starts[run_idx]
    grp = (half * NBLK + blk)[order]
    pos = tile_off[grp] * TILE_E + rank  # position within the core's edge array
    corev = core[order]

    kv_idx = np.zeros((M, TT * TILE_E), np.int64)
    dloc = np.full((M, TT * TILE_E), -1, np.int64)
    kv_idx[corev, pos] = (src_p - half * HALF)[order]
    dloc[corev, pos] = (dst_l - blk * BLK)[order]
    assert kv_idx.max() < 2 ** 15

    S = np.zeros((M, 128, TT * BLK), BNP)
    ST = np.zeros((M, 128, TT * BLK), BNP)
    dd = dloc.reshape(M, TT, TILE_E)
    mm, tt, pp = np.nonzero(dd >= 0)
    dv = dd[mm, tt, pp]
    S[mm, pp, tt * BLK + dv] = 1.0
    ST[mm, dv, tt * BLK + pp] = 1.0

    kv_w = _wrap_idx(kv_idx)

    # node features, transposed + padded (bf16)
    xT_pad = np.zeros((D_IN, NPAD), np.float32)
    n_ids = np.arange(N)
    pid = (n_ids // NPC_REAL) * NPC + (n_ids % NPC_REAL)
    xT_pad[:, pid] = np.asarray(x).T
    xT = xT_pad.astype(BNP)

    wt = {}
    for L in (1, 2, 3):
        W_all, b_all = weights[L]   # packed k|v|q|s by _weights_from_inputs
        if L == 1:
            wt["W1"] = W_all.astype(BNP)
            wt["brep1"] = np.ascontiguousarray(
                np.tile(b_all[None, :], (128, 1)).astype(np.float32))
        else:
            wt[f"W{L}"] = np.concatenate(
                [W_all, b_all[None, :]], 0).astype(BNP)

    in_maps = []
    for m in range(M):
        im = dict(
            xT=np.ascontiguousarray(xT),
            xoT=np.ascontiguousarray(xT[:, m * NPC:(m + 1) * NPC]),
            kvidx=np.ascontiguousarray(kv_w[m]),
            S_in=np.ascontiguousarray(S[m]),
            ST_in=np.ascontiguousarray(ST[m]),
            **wt,
        )
        in_maps.append(im)

    # tile metadata: (half, blk, start, stop)
    meta = []
    groups = []  # (f, b, t0, T)
    t = 0
    for f in (0, 1):
        for b in range(NBLK):
            T = int(tiles[f, b])
            groups.append((f, b, t, T))
            t += T
            for i in range(T):
                meta.append((f, b, i == 0, i == T - 1))
    # chunks = runs of whole groups (same half), <= CHUNK_T tiles, so each
    # chunk ends exactly at a group boundary (trailing pads are call-trailing)
    chunks = []
    gi = 0
    while gi < len(groups):
        f, b, t0, T = groups[gi]
        nt = T
        gj = gi + 1
        while (gj < len(groups) and groups[gj][0] == f
               and nt + groups[gj][3] <= CHUNK_T):
            nt += groups[gj][3]
            gj += 1
        chunks.append((t0, nt, f, groups[gj - 1][1]))  # last block of chunk
        gi = gj

    # per-(core, chunk) valid gather counts: for chunks beyond the warmup,
    # the final group's padding slots get idx -1 (skipped by the gather)
    nch = len(chunks)
    gcnt = np.zeros((M, nch), np.int32)
    for j, (t0, nt, f, b_last) in enumerate(chunks):
        n = nt * TILE_E
        if j < WARM_CH:
            gcnt[:, j] = n
            continue
        Tg = int(tiles[f, b_last])
        g_t0 = t0 + nt - Tg          # first tile of the final group
        for m in range(M):
            cm = int(counts[m, f, b_last])
            pad = Tg * TILE_E - cm
            gcnt[m, j] = n - pad
            if pad:
                kv_idx_w_set_m1(kv_w, m, g_t0 * TILE_E + cm, (t0 + nt) * TILE_E)
    for m in range(M):
        in_maps[m]["kvidx"] = np.ascontiguousarray(kv_w[m])
        in_maps[m]["gcnt"] = np.ascontiguousarray(gcnt[m:m + 1])
    return in_maps, dict(TT=TT, meta=meta, chunks=chunks, tiles=tiles)


def kv_idx_w_set_m1(kv_w, m, lo, hi):
    """Set wrapped-index positions [lo, hi) to -1 for core m."""
    js = np.arange(lo, hi)
    rows = (np.arange(8) * 16)[:, None] + (js % 16)[None, :]
    cols = np.broadcast_to(js // 16, rows.shape)
    kv_w[m, rows, cols] = -1


def build_module(d, meta):
    TT, chunks, tmeta = meta["TT"], meta["chunks"], meta["meta"]
    M, NPC, NPAD, HALF, NBLK, PA_CHUNK = (
        d["M"], d["NPC"], d["NPAD"], d["HALF"], d["NBLK"], d["PA_CHUNK"])
    D_IN, F, KV, WC, HID, H, C, RHSW = (
        d["D_IN"], d["F"], d["KV"], d["WC"], d["HID"], d["H"], d["C"], d["RHSW"])
    NT_ALL = NPAD // 128          # node tiles, all cores
    NCH_ALL = NT_ALL // PA_CHUNK  # phase A chunks
    RANK_CH = NBLK // PA_CHUNK    # chunks per rank slab
    CA = max(0, RANK_CH - 1)      # rank-slab chunks covered by first collective
    SPLIT_B = CA * PA_CHUNK       # blocks covered by first collective
    COLA = SPLIT_B * 128

    nc = bacc.Bacc("TRN2", target_bir_lowering=False, debug=False, num_devices=M)
    inp = {}
    for name, shape, dt in [
        ("xT", [D_IN, NPAD], BF16), ("xoT", [D_IN, NPC], BF16),
        ("W1", [D_IN, WC], BF16), ("brep1", [128, WC], F32),
        ("W2", [HID + 1, WC], BF16), ("W3", [HID + 1, WC], BF16),
        ("kvidx", [128, TT * 8], I16),
        ("gcnt", [1, len(chunks)], mybir.dt.int32),
        ("S_in", [128, TT * BLK], BF16), ("ST_in", [128, TT * BLK], BF16),
    ]:
        inp[name] = nc.dram_tensor(name, shape, dt, kind="ExternalInput")
    h_out = nc.dram_tensor("h_out", [NPC, HID], F32, kind="ExternalOutput")

    with tile.TileContext(nc) as tc:
        with tc.tile_pool(name="dram", bufs=1, space="DRAM") as dram, \
             tc.tile_pool(name="res", bufs=1) as res:
            kv_lo = dram.tile([HALF, KV], BF16)
            kv_hi = dram.tile([NPAD - HALF, KV], BF16)
            if CA > 0:
                coll_inA = dram.tile([HID + 1, COLA], BF16)
                coll_outA = dram.tile([M * (HID + 1), COLA], BF16)
            coll_inB = dram.tile([HID + 1, NPC - COLA], BF16)
            coll_outB = dram.tile([M * (HID + 1), NPC - COLA], BF16)

            nc.gpsimd.load_library(library_config.mlp)

            # resident SBUF
            W1_sb = res.tile([D_IN, WC], BF16)
            brep1_sb = res.tile([128, WC], F32)
            W2_sb = res.tile([HID + 1, WC], BF16)
            W3_sb = res.tile([HID + 1, WC], BF16)
            kvidx_sb = res.tile([128, TT * 8], I16)
            gcnt_sb = res.tile([1, len(chunks)], mybir.dt.int32)
            q_sb = res.tile([128, NBLK * F], BF16)
            s_sb = res.tile([128, NBLK * HID], F32)
            hTown = res.tile([HID + 1, NPC], BF16)
            partA = res.tile([128, NBLK * RHSW], F32)
            ident = res.tile([128, 128], F32)
            eps2 = res.tile([128, H], F32)

            for sb, t in ((W1_sb, "W1"), (brep1_sb, "brep1"), (W2_sb, "W2"),
                          (W3_sb, "W3"), (kvidx_sb, "kvidx"), (gcnt_sb, "gcnt")):
                nc.sync.dma_start(sb[:], inp[t].ap())
            make_identity(nc, ident[:])
            nc.vector.memset(hTown[HID:HID + 1, :], 1.0)
            nc.vector.memset(eps2[:], H * 1e-16)

            for layer in (1, 2, 3):
                W_sb = {1: None, 2: W2_sb, 3: W3_sb}[layer]

                # ---------- Phase A-kv: kv table for ALL nodes ----------
                with tc.tile_pool(name="pa", bufs=4) as pa, \
                     tc.tile_pool(name="pap", bufs=4, space="PSUM") as pap:

                    # kv for all nodes. Order: lo table half first; within a
                    # half, chunks covered by the first (already-landed)
                    # collective before chunks needing the second one.
                    if layer == 1:
                        ch_list = list(range(NCH_ALL))
                    else:
                        ch_list = []
                        for rg in (range(0, M // 2), range(M // 2, M)):
                            for cg in (range(0, CA), range(CA, RANK_CH)):
                                for r in rg:
                                    for c in cg:
                                        ch_list.append(r * RANK_CH + c)
                    for ch in ch_list:
                        cols = slice(ch * PA_CHUNK * 128, (ch + 1) * PA_CHUNK * 128)
                        if layer == 1:
                            la = pa.tile([D_IN, PA_CHUNK * 128], BF16, tag="la")
                            nc.sync.dma_start(la[:], inp["xT"].ap()[:, cols])
                        else:
                            r, c = ch // RANK_CH, ch % RANK_CH
                            la = pa.tile([HID + 1, PA_CHUNK * 128], BF16, tag="la")
                            if c < CA:
                                src = coll_outA[r * (HID + 1):(r + 1) * (HID + 1),
                                                c * PA_CHUNK * 128:
                                                (c + 1) * PA_CHUNK * 128]
                            else:
                                src = coll_outB[r * (HID + 1):(r + 1) * (HID + 1),
                                                (c - CA) * PA_CHUNK * 128:
                                                (c - CA + 1) * PA_CHUNK * 128]
                            nc.sync.dma_start(la[:], src)
                        kvst = pa.tile([128, PA_CHUNK * KV], BF16, tag="kvst")
                        for t in range(PA_CHUNK):
                            ps = pap.tile([128, KV], F32, tag="pskv", name="pskv")
                            dstp = kvst[:, t * KV:(t + 1) * KV]
                            if layer == 1:
                                nc.tensor.matmul(ps[:], la[:, t * 128:(t + 1) * 128],
                                                 W1_sb[:, 0:KV],
                                                 start=True, stop=True)
                                nc.vector.tensor_tensor(
                                    dstp, ps[:], brep1_sb[:, 0:KV], op=OP.add)
                            else:
                                nc.tensor.matmul(ps[:], la[:, t * 128:(t + 1) * 128],
                                                 W_sb[:, 0:KV], start=True, stop=True)
                                if t % 2 == 0:
                                    nc.scalar.copy(dstp, ps[:])
                                else:
                                    nc.vector.tensor_copy(dstp, ps[:])
                        row0 = ch * PA_CHUNK * 128
                        tgt = (kv_lo[row0:row0 + PA_CHUNK * 128, :]
                               if row0 < HALF else
                               kv_hi[row0 - HALF:row0 - HALF + PA_CHUNK * 128, :])
                        nc.sync.dma_start(
                            tgt.rearrange("(t p) e -> p t e", p=128),
                            kvst[:].rearrange("p (t e) -> p t e", e=KV))

                    # own q/s (phase B needs q_sb only after its first gather lands)
                    for ch in range(RANK_CH):
                        cols = slice(ch * PA_CHUNK * 128, (ch + 1) * PA_CHUNK * 128)
                        if layer == 1:
                            la = pa.tile([D_IN, PA_CHUNK * 128], BF16, tag="la")
                            nc.sync.dma_start(la[:], inp["xoT"].ap()[:, cols])
                        for t in range(PA_CHUNK):
                            gt = ch * PA_CHUNK + t  # own node tile index
                            ps = pap.tile([128, F + HID], F32, tag="psqs",
                                          name="psqs")
                            if layer == 1:
                                nc.tensor.matmul(ps[:], la[:, t * 128:(t + 1) * 128],
                                                 W1_sb[:, KV:WC],
                                                 start=True, stop=True)
                                nc.vector.tensor_tensor(
                                    q_sb[:, gt * F:(gt + 1) * F], ps[:, 0:F],
                                    brep1_sb[:, KV:KV + F], op=OP.add)
                                nc.vector.tensor_tensor(
                                    s_sb[:, gt * HID:(gt + 1) * HID],
                                    ps[:, F:F + HID],
                                    brep1_sb[:, KV + F:WC], op=OP.add)
                            else:
                                nc.tensor.matmul(
                                    ps[:],
                                    hTown[:, gt * 128:(gt + 1) * 128],
                                    W_sb[:, KV:WC], start=True, stop=True)
                                nc.scalar.copy(q_sb[:, gt * F:(gt + 1) * F],
                                               ps[:, 0:F])
                                nc.vector.tensor_copy(
                                    s_sb[:, gt * HID:(gt + 1) * HID],
                                    ps[:, F:F + HID])

                # ---------- Phase B: edges ----------
                with tc.tile_pool(name="pb", bufs=4) as pb, \
                     tc.tile_pool(name="pb1", bufs=3) as pb1, \
                     tc.tile_pool(name="pbp", bufs=3, space="PSUM") as pbp, \
                     tc.tile_pool(name="qep", bufs=4, space="PSUM") as qep, \
                     tc.tile_pool(name="epp", bufs=1, space="PSUM") as epp, \
                     tc.tile_pool(name="ep", bufs=2) as ep:
                    psum_blk = {}
                    for j, (t0, nt, fhalf, _blast) in enumerate(chunks):
                        n = nt * TILE_E
                        nreg = n
                        kvg = pb.tile([128, CHUNK_T, KV], BF16, tag="kvg")
                        Sg = pb1.tile([128, CHUNK_T * BLK], BF16, tag="Sg")
                        STg = pb1.tile([128, CHUNK_T * BLK], BF16, tag="STg")
                        prod = pb1.tile([128, CHUNK_T * F], F32, tag="prod", bufs=2)
                        alph = pb1.tile([128, CHUNK_T * H], F32, tag="alph")
                        rhs = pb.tile([128, CHUNK_T, RHSW], BF16, tag="rhs")

                        in_ap = kv_lo[:] if fhalf == 0 else kv_hi[:]
                        nc.gpsimd.dma_gather(
                            out_ap=kvg[:, 0:nt, :], in_ap=in_ap,
                            idxs_ap=kvidx_sb[:, t0 * 8:t0 * 8 + nt * 8],
                            num_idxs=n, num_idxs_reg=nreg, elem_size=KV,
                            single_packet=False)
                        nc.sync.dma_start(
                            Sg[:, 0:n], inp["S_in"].ap()[:, t0 * BLK:t0 * BLK + n])
                        nc.sync.dma_start(
                            STg[:, 0:n], inp["ST_in"].ap()[:, t0 * BLK:t0 * BLK + n])

                        # q_edges per tile via one-hot broadcast matmul, then
                        # prod = (q_edges * 1/sqrt(C)) * k
                        for i in range(nt):
                            tg = t0 + i
                            b = tmeta[tg][1]
                            qe = qep.tile([128, F], F32, name="qe", tag="qe")
                            nc.tensor.matmul(
                                qe[:], STg[:, i * BLK:(i + 1) * BLK],
                                q_sb[:, b * F:(b + 1) * F], start=True, stop=True)
                            nc.vector.scalar_tensor_tensor(
                                out=prod[:, i * F:(i + 1) * F],
                                in0=qe[:], scalar=float(1.0 / np.sqrt(C)),
                                in1=kvg[:, i, 0:F], op0=OP.mult, op1=OP.mult)
                        # alpha[p, t, h] = sum_c prod
                        nc.vector.reduce_sum(
                            alph[:, 0:nt * H].rearrange("p (t h) -> p t h", h=H),
                            prod[:, 0:nt * F].rearrange(
                                "p (t h c) -> p t h c", h=H, c=C),
                            axis=mybir.AxisListType.X)
                        # ex = exp(alpha) -> rhs[:, :, 0:H]
                        nc.scalar.activation(
                            rhs[:, 0:nt, 0:H],
                            alph[:, 0:nt * H].rearrange("p (t h) -> p t h", h=H),
                            AF.Exp)
                        # exv = v * ex -> rhs[:, :, H:]
                        nc.vector.tensor_tensor(
                            out=rhs[:, 0:nt, H:RHSW].rearrange(
                                "p t (h c) -> p t h c", c=C),
                            in0=kvg[:, 0:nt, F:KV].rearrange(
                                "p t (h c) -> p t h c", c=C),
                            in1=rhs[:, 0:nt, 0:H].to_broadcast([128, nt, H, C]),
                            op=OP.mult)

                        for i in range(nt):
                            tg = t0 + i
                            f, b, st, sp = tmeta[tg]
                            if st:
                                psum_blk[(f, b)] = pbp.tile(
                                    [128, RHSW], F32, name="pblk", tag="pblk")
                            nc.tensor.matmul(
                                psum_blk[(f, b)][:],
                                Sg[:, i * BLK:(i + 1) * BLK],
                                rhs[:, i, :], start=st, stop=sp)
                            if not sp:
                                continue
                            ps = psum_blk.pop((f, b))
                            pa_sl = partA[:, b * RHSW:(b + 1) * RHSW]
                            if f == 0:
                                nc.scalar.copy(pa_sl, ps[:])
                                continue
                            # ---- epilogue for block b ----
                            tot = ep.tile([128, RHSW], F32, tag="tot")
                            nc.vector.tensor_tensor(tot[:], ps[:], pa_sl, op=OP.add)
                            # rec = (1/H) / (denom + 1e-16), via 1/(H*denom + H*1e-16)
                            rec = ep.tile([128, H], F32, tag="rec")
                            nc.vector.scalar_tensor_tensor(
                                out=rec[:], in0=tot[:, 0:H], scalar=float(H),
                                in1=eps2[:], op0=OP.mult, op1=OP.add)
                            nc.vector.reciprocal(rec[:], rec[:])
                            m0 = ep.tile([128, C], F32, tag="m0")
                            nc.vector.scalar_tensor_tensor(
                                out=m0[:], in0=tot[:, H:H + C],
                                scalar=rec[:, 0:1],
                                in1=s_sb[:, b * HID:(b + 1) * HID],
                                op0=OP.mult, op1=OP.add)
                            hp2 = ep.tile([128, HID], F32, tag="hp2")
                            nc.vector.scalar_tensor_tensor(
                                out=hp2[:], in0=tot[:, H + C:H + 2 * C],
                                scalar=rec[:, 1:2], in1=m0[:],
                                op0=OP.mult, op1=OP.add)
                            hblk = ep.tile([128, HID], F32, tag="hblk")
                            nc.scalar.activation(hblk[:], hp2[:], AF.Relu)
                            if layer < 3:
                                pst = epp.tile([HID, 128], F32)
                                nc.tensor.transpose(pst[:], hblk[:], ident[:])
                                nc.vector.tensor_copy(
                                    hTown[0:HID, b * 128:(b + 1) * 128], pst[:])
                                if CA > 0 and b == SPLIT_B - 1:
                                    # first half of h^T is final: allgather it
                                    # while the rest of phase B runs
                                    nc.sync.dma_start(coll_inA[:, :],
                                                      hTown[:, 0:COLA])
                                    nc.gpsimd.collective_compute(
                                        "AllGather", OP.bypass,
                                        ins=[coll_inA.opt()],
                                        outs=[coll_outA.opt()],
                                        replica_groups=[list(range(M))])
                            else:
                                nc.sync.dma_start(
                                    h_out.ap()[b * 128:(b + 1) * 128, :], hblk[:])
                    assert not psum_blk

                if layer < 3:
                    nc.sync.dma_start(coll_inB[:, :], hTown[:, COLA:])
                    nc.gpsimd.collective_compute(
                        "AllGather", OP.bypass,
                        ins=[coll_inB.opt()], outs=[coll_outB.opt()],
                        replica_groups=[list(range(M))])
    nc.compile()
    return nc


# ---------------- public entry ----------------
_CACHE = {}


def _weights_from_inputs(inputs, d):
    # packed column order: k | v | q | s
    wt = {}
    for L in (1, 2, 3):
        W_all = np.concatenate(
            [np.asarray(inputs[f"W{L}{nm}"], np.float32) for nm in ("k", "v", "q", "s")],
            axis=1)
        b_all = np.concatenate(
            [np.asarray(inputs[f"b{L}{nm}"], np.float32) for nm in ("k", "v", "q", "s")])
        wt[L] = (W_all, b_all)
    return wt


def _install_ntff_shim():
    import types
    if "antenv.axon_hooks" in sys.modules:
        return
    try:
        from trn_agent_boot.trn_boot import _ntff_profile_via_ctypes
        hook = _ntff_profile_via_ctypes("/opt/axon/libaxon_pjrt.so")
    except Exception:
        hook = None
    mod = types.ModuleType("antenv.axon_hooks")
    mod.get_axon_ntff_profile_hook = lambda: hook
    mod.set_axon_ntff_profile_hook = lambda h: None
    sys.modules["antenv.axon_hooks"] = mod
    try:
        import antenv
        antenv.axon_hooks = mod
    except Exception:
        pass


def run(inputs, cfg=SPEC, trace=False):
    d = _derive(cfg)
    wt = _weights_from_inputs(inputs, d)
    in_maps, meta = _prep(inputs["x"], inputs["edge_index"], wt, d)
    key = (tuple(sorted(cfg.items())), meta["TT"],
           tuple(tuple(r) for r in meta["tiles"]))
    if key not in _CACHE:
        _CACHE[key] = build_module(d, meta)
    nc = _CACHE[key]
    if trace:
        _install_ntff_shim()
    res = bass_utils.run_bass_kernel_spmd(
        nc, in_maps, core_ids=list(range(d["M"])), trace=trace)
    outs = [res.results[m]["h_out"][:d["NPC_REAL"]] for m in range(d["M"])]
    full = np.concatenate(outs, axis=0).astype(np.float32)
    return full, res


def kernel(**inputs) -> np.ndarray:
    trace = bool(os.environ.get("KERNEL_TRACE"))
    full, res = run(inputs, SPEC, trace=trace)
    if trace and res.exec_time_ns is not None:
        print(f"HW exec time: {res.exec_time_ns} ns")
    return full
